# revision 1
# baseline (speedup 1.0000x reference)
"""Trainium2 Bass kernel for nn_Block_52527450030210 (dense transformer block).

B=8, S=1024, E=1024, H=16 heads (D=64), F=4096. Data-parallel: batch element i
runs on core i (no collectives). Matmuls in bf16 with fp32 PSUM accumulation;
LayerNorms/softmax in fp32. Softmax is over the QUERY axis (dim=-2), so scores
are computed transposed ([k, q] layout) making the softmax reduction a
free-axis reduction, and 1/Z folds into v (Z is per contraction-index k).

Self-contained: hardcodes shapes, includes the walrus single-sync-wait
workaround (this container's walrus accepts only one sync-wait per
instruction; Tile emits several, so extra waits are hoisted onto same-engine
NoOps).
"""

import numpy as np
import ml_dtypes

import concourse.bass as bass
import concourse.mybir as mybir
import concourse.tile as tile
from concourse.bass_utils import run_bass_kernel_spmd
from concourse.masks import make_identity
from concourse.vector_clock import ScopedClock
from contextlib import ExitStack

F32 = mybir.dt.float32
BF16 = mybir.dt.bfloat16
AF = mybir.ActivationFunctionType
ALU = mybir.AluOpType

B, S, E, H, D, F = 8, 1024, 1024, 16, 64, 4096
P = 128
NE = E // P   # 8 e-chunks
NS = S // P   # 8 s-tiles
NF = F // P   # 32 f-tiles
EPS = 1e-5

# ---------------------------------------------------------------- waitfix ---

_wf_counter = [0]


def _wait_nop(nc, engine, wait, debug):
    _wf_counter[0] += 1
    nop = mybir.InstNoOp(
        name=f"I-wsplit-{_wf_counter[0]}", ins=[], outs=[], debug=debug,
        bass_nofuse=True,
    )
    nop.engine = engine
    nop.sync_info = mybir.SyncInfo(on_wait=[wait], on_update=[])
    nc.register_instruction(nop, overwrite=True)
    return nop


def _split_sync_waits(nc):
    for _name, bb in nc.bb_map.items():
        if not hasattr(bb, "instructions"):
            bb = bb.bb
        il = bb.instructions
        changed = False
        new = []
        for inst in il:
            si = inst.sync_info
            if si is not None and si.on_wait and len(si.on_wait) > 1:
                waits = list(si.on_wait)
                for w in waits[:-1]:
                    new.append(_wait_nop(nc, inst.engine, w, inst.debug))
                si.on_wait = waits[-1:]
                changed = True
            new.append(inst)
        if changed:
            bb.instructions = new


def _patched_drain_and_barrier(self, tick_clock, wait_clock):
    nop0 = self.nc.sync.nop(nofuse=True, hint="tail_wait")
    wait_clock.add_sem_waits(nop0.ins, ScopedClock({None: tick_clock.global_clock}))
    si = nop0.ins.sync_info
    waits = list(si.on_wait) if si and si.on_wait else []
    if len(waits) > 1:
        si.on_wait = waits[:1]
        rest = waits[1:]
        while rest:
            nop = self.nc.sync.nop(nofuse=True, hint="tail_wait")
            nop.ins.sync_info = mybir.SyncInfo(on_wait=rest[:1], on_update=[])
            rest = rest[1:]
    self.nc.sync.drain()
    self.nc.all_engine_barrier()
    assert self.sems is not None
    popped = self.nc._tile_sem_poison_stack.pop()
    assert popped is self._sem_poison
    self.nc.clear_and_free_semaphores(list(self.sems.allocated().values()))
    self.nc.all_engine_barrier()


tile.TileContext._drain_and_barrier = _patched_drain_and_barrier

# ----------------------------------------------------------------- build -----


def _bcast_ap(dram_t, n):
    """AP that DMA-broadcasts a [n] DRAM vector to [128, n] (partition step 0)."""
    return bass.AP(tensor=dram_t, offset=0, ap=[[0, P], [1, n]])


def build(reps=1, mode="full"):
    nc = bass.Bass()

    xT = nc.dram_tensor("xT", [E, S], BF16, kind="ExternalInput")
    x_f = nc.dram_tensor("x", [S, E], F32, kind="ExternalInput")
    wq = nc.dram_tensor("wq", [E, E], BF16, kind="ExternalInput")
    wk = nc.dram_tensor("wk", [E, E], BF16, kind="ExternalInput")
    wv = nc.dram_tensor("wv", [E, E], BF16, kind="ExternalInput")
    w1 = nc.dram_tensor("w1", [E, F], BF16, kind="ExternalInput")
    w2 = nc.dram_tensor("w2", [F, E], BF16, kind="ExternalInput")
    bq = nc.dram_tensor("bq", [E], F32, kind="ExternalInput")
    bk = nc.dram_tensor("bk", [E], F32, kind="ExternalInput")
    bv = nc.dram_tensor("bv", [E], F32, kind="ExternalInput")
    b1 = nc.dram_tensor("b1", [F], F32, kind="ExternalInput")
    b2 = nc.dram_tensor("b2", [E], F32, kind="ExternalInput")
    g1 = nc.dram_tensor("g1", [E], F32, kind="ExternalInput")
    be1 = nc.dram_tensor("be1", [E], F32, kind="ExternalInput")
    gff = nc.dram_tensor("gff", [E], F32, kind="ExternalInput")
    bff = nc.dram_tensor("bff", [E], F32, kind="ExternalInput")
    g2 = nc.dram_tensor("g2", [E], F32, kind="ExternalInput")
    be2 = nc.dram_tensor("be2", [E], F32, kind="ExternalInput")

    xT_c = xT.rearrange("(c p) s -> c p s", p=P)
    x_c = x_f.rearrange("(m p) e -> m p e", p=P)
    wq_c = wq.rearrange("(c p) e -> c p e", p=P)
    wk_c = wk.rearrange("(c p) e -> c p e", p=P)
    wv_c = wv.rearrange("(c p) e -> c p e", p=P)
    w1_c = w1.rearrange("(c p) f -> c p f", p=P)
    w2_c = w2.rearrange("(c p) e -> c p e", p=P)

    with tile.TileContext(nc) as tc:
      for rep in range(reps):
        out_d = nc.dram_tensor("out" if rep == 0 else f"out_r{rep}",
                               [S, E], F32, kind="ExternalOutput")
        out_c = out_d.rearrange("(m p) e -> m p e", p=P)
        adn1_d = nc.dram_tensor(f"adn1_d_{rep}", [S, E], F32)
        adn1d_c = adn1_d.rearrange("(m p) e -> m p e", p=P)
        with ExitStack() as top:
            const = top.enter_context(tc.tile_pool(name="const", bufs=1))
            ident = const.tile([P, P], BF16)
            make_identity(nc, ident)
            eps_t = const.tile([P, 1], F32)
            nc.vector.memset(eps_t[:], EPS)
            bq_sb = const.tile([P, NE], F32)
            bk_sb = const.tile([P, NE], F32)
            b1_sb = const.tile([P, NF], F32)
            nc.sync.dma_start(bq_sb[:], bq.rearrange("(c p) -> p c", p=P))
            nc.sync.dma_start(bk_sb[:], bk.rearrange("(c p) -> p c", p=P))
            nc.sync.dma_start(b1_sb[:], b1.rearrange("(c p) -> p c", p=P))

            adn1T_pool = top.enter_context(tc.tile_pool(name="adn1T", bufs=NE))
            adn1T = [adn1T_pool.tile([P, S], BF16, tag="adn1T", name=f"r{rep}_adn1T_{i}")
                     for i in range(NE)]

            if mode != "ffn":
                with ExitStack() as attsb_scope:
                    attsb_pool = attsb_scope.enter_context(
                        tc.tile_pool(name="attsb", bufs=NS))
                    att_sb = [attsb_pool.tile([P, E], BF16, tag="att", name=f"r{rep}_att_{i}")
                              for i in range(NS)]

                    # -------------------------------------------- phase 1: qkv + attn
                    with ExitStack() as ph1:
                        xT_pool = ph1.enter_context(tc.tile_pool(name="xTp", bufs=NE))
                        w_pool = ph1.enter_context(tc.tile_pool(name="wp", bufs=NE))
                        v_pool = ph1.enter_context(tc.tile_pool(name="vp", bufs=NS))
                        qk_pool = ph1.enter_context(tc.tile_pool(name="qkp", bufs=4))
                        exp_pool = ph1.enter_context(tc.tile_pool(name="expp", bufs=8))
                        small_pool = ph1.enter_context(
                            tc.tile_pool(name="smallp", bufs=16))
                        attT_pool = ph1.enter_context(
                            tc.tile_pool(name="attTp", bufs=3))
                        bv_pool = ph1.enter_context(tc.tile_pool(name="bvp", bufs=1))
                        ps_sc = ph1.enter_context(
                            tc.tile_pool(name="ps_sc", bufs=4, space="PSUM"))
                        ps_av = ph1.enter_context(
                            tc.tile_pool(name="ps_av", bufs=2, space="PSUM"))

                        bv_b = bv_pool.tile([P, E], F32)
                        nc.gpsimd.dma_start(bv_b[:], _bcast_ap(bv, E))

                        xT_sb = []
                        wq_sb, wk_sb, wv_sb = [], [], []
                        for c in range(NE):
                            t = xT_pool.tile([P, S], BF16, tag="xT", name=f"r{rep}_xT_{c}")
                            nc.sync.dma_start(t[:], xT_c[c])
                            xT_sb.append(t)
                        for c in range(NE):
                            tq = w_pool.tile([P, E], BF16, tag="wq", name=f"r{rep}_wq_{c}")
                            tk = w_pool.tile([P, E], BF16, tag="wk", name=f"r{rep}_wk_{c}")
                            tv = w_pool.tile([P, E], BF16, tag="wv", name=f"r{rep}_wv_{c}")
                            nc.sync.dma_start(tq[:], wq_c[c])
                            nc.sync.dma_start(tk[:], wk_c[c])
                            nc.sync.dma_start(tv[:], wv_c[c])
                            wq_sb.append(tq)
                            wk_sb.append(tk)
                            wv_sb.append(tv)

                        # v[s, e] = x @ Wv + bv
                        v_sb = []
                        for m in range(NS):
                            vt = v_pool.tile([P, E], BF16, tag="v", name=f"r{rep}_v_{m}")
                            for n in range(2):
                                ps = ps_sc.tile([P, 512], F32, tag="sc",
                                                name=f"r{rep}_psv_{m}_{n}")
                                for c in range(NE):
                                    nc.tensor.matmul(
                                        ps[:],
                                        xT_sb[c][:, m * P:(m + 1) * P],
                                        wv_sb[c][:, n * 512:(n + 1) * 512],
                                        start=(c == 0), stop=(c == NE - 1))
                                nc.vector.tensor_tensor(
                                    vt[:, n * 512:(n + 1) * 512], ps[:],
                                    bv_b[:, n * 512:(n + 1) * 512], ALU.add)
                            v_sb.append(vt)

                        # per e-chunk j: qT_j, kT_j then attention head pair (2j,2j+1)
                        for j in range(NE):
                            qt = qk_pool.tile([P, S], BF16, tag="qT", name=f"r{rep}_qT_{j}")
                            kt = qk_pool.tile([P, S], BF16, tag="kT", name=f"r{rep}_kT_{j}")
                            for di, (dst, w_sb, bias) in enumerate(
                                    ((qt, wq_sb, bq_sb), (kt, wk_sb, bk_sb))):
                                for n in range(2):
                                    ps = ps_sc.tile([P, 512], F32, tag="sc",
                                                    name=f"r{rep}_psqk_{j}_{di}_{n}")
                                    for c in range(NE):
                                        nc.tensor.matmul(
                                            ps[:],
                                            w_sb[c][:, j * P:(j + 1) * P],
                                            xT_sb[c][:, n * 512:(n + 1) * 512],
                                            start=(c == 0), stop=(c == NE - 1))
                                    nc.vector.tensor_scalar_add(
                                        dst[:, n * 512:(n + 1) * 512], ps[:],
                                        bias[:, j:j + 1])

                            avp = ps_av.tile([P, 1024], F32, tag="av", name=f"r{rep}_av_{j}")
                            for kc in range(NS):
                                expts = []
                                for h in range(2):
                                    o = h * 64
                                    expt = exp_pool.tile([P, 1024], BF16, tag="expt",
                                                         name=f"r{rep}_ex_{j}_{kc}_{h}")
                                    zt = small_pool.tile([P, 1], F32, tag="z",
                                                         name=f"r{rep}_z_{j}_{kc}_{h}")
                                    z1 = small_pool.tile([P, 1], F32, tag="z1",
                                                         name=f"r{rep}_z1_{j}_{kc}_{h}")
                                    for n in range(2):
                                        scps = ps_sc.tile([P, 512], F32, tag="sc",
                                                          name=f"r{rep}_sc_{j}_{kc}_{h}_{n}")
                                        nc.tensor.matmul(
                                            scps[:],
                                            kt[o:o + 64, kc * P:(kc + 1) * P],
                                            qt[o:o + 64, n * 512:(n + 1) * 512],
                                            start=True, stop=True,
                                            tile_position=(o, 0))
                                        nc.scalar.activation(
                                            expt[:, n * 512:(n + 1) * 512],
                                            scps[:], AF.Exp, scale=0.1,
                                            accum_out=(zt[:] if n == 0 else z1[:]))
                                    nc.vector.tensor_tensor(zt[:], zt[:], z1[:],
                                                            ALU.add)
                                    nc.vector.reciprocal(zt[:], zt[:])
                                    vsc = small_pool.tile([P, 64], BF16, tag="vsc",
                                                          name=f"r{rep}_vs_{j}_{kc}_{h}")
                                    head = 2 * j + h
                                    nc.vector.tensor_scalar_mul(
                                        vsc[:],
                                        v_sb[kc][:, head * 64:(head + 1) * 64],
                                        zt[:])
                                    expts.append((o, expt, vsc))
                                for o, expt, vsc in expts:
                                    for n in range(2):
                                        nc.tensor.matmul(
                                            avp[o:o + 64, n * 512:(n + 1) * 512],
                                            vsc[:], expt[:, n * 512:(n + 1) * 512],
                                            start=(kc == 0), stop=(kc == NS - 1),
                                            tile_position=(0, o))

                            attj = attT_pool.tile([P, S], BF16, tag="attT",
                                                  name=f"r{rep}_attT_{j}")
                            nc.vector.tensor_copy(attj[:], avp[:])
                            for m in range(NS):
                                tp = ps_av.tile([P, P], BF16, tag="av",
                                                name=f"r{rep}_tp_{j}_{m}")
                                nc.tensor.transpose(tp[:], attj[:, m * P:(m + 1) * P],
                                                    ident[:])
                                nc.scalar.copy(att_sb[m][:, j * P:(j + 1) * P], tp[:])

                    # ------------------------------------------ phase 2: LN1 + adn1T
                    with ExitStack() as ph2:
                        gb_pool = ph2.enter_context(tc.tile_pool(name="gbp", bufs=1))
                        x_pool = ph2.enter_context(tc.tile_pool(name="xp", bufs=4))
                        u_pool = ph2.enter_context(tc.tile_pool(name="up", bufs=3))
                        a1_pool = ph2.enter_context(tc.tile_pool(name="a1p", bufs=3))
                        st_pool = ph2.enter_context(tc.tile_pool(name="stp", bufs=8))
                        abf_pool = ph2.enter_context(tc.tile_pool(name="abfp", bufs=3))
                        ps_t = ph2.enter_context(
                            tc.tile_pool(name="ps_t", bufs=2, space="PSUM"))

                        g1_b = gb_pool.tile([P, E], F32, tag="g1")
                        be1_b = gb_pool.tile([P, E], F32, tag="be1")
                        nc.gpsimd.dma_start(g1_b[:], _bcast_ap(g1, E))
                        nc.gpsimd.dma_start(be1_b[:], _bcast_ap(be1, E))

                        for m in range(NS):
                            xt = x_pool.tile([P, E], F32, tag="x", name=f"r{rep}_x_{m}")
                            nc.sync.dma_start(xt[:], x_c[m])
                            stats = st_pool.tile([P, 2, 6], F32, tag="st",
                                                 name=f"r{rep}_st_{m}")
                            mv = st_pool.tile([P, 2], F32, tag="mv", name=f"r{rep}_mv_{m}")
                            nc.vector.bn_stats(stats[:, 0, :], att_sb[m][:, 0:512])
                            nc.vector.bn_stats(stats[:, 1, :], att_sb[m][:, 512:1024])
                            nc.vector.bn_aggr(mv[:], stats[:])
                            rstd = st_pool.tile([P, 1], F32, tag="rstd",
                                                name=f"r{rep}_rstd_{m}")
                            nc.scalar.activation(rstd[:], mv[:, 1:2], AF.Sqrt,
                                                 bias=eps_t[:])
                            nc.vector.reciprocal(rstd[:], rstd[:])
                            u = u_pool.tile([P, E], F32, tag="u", name=f"r{rep}_u_{m}")
                            nc.vector.tensor_scalar(u[:], att_sb[m][:], mv[:, 0:1],
                                                    rstd[:], ALU.subtract, ALU.mult)
                            nc.vector.tensor_tensor(u[:], u[:], g1_b[:], ALU.mult)
                            nc.vector.tensor_tensor(u[:], u[:], be1_b[:], ALU.add)
                            a1 = a1_pool.tile([P, E], F32, tag="a1", name=f"r{rep}_a1_{m}")
                            nc.vector.tensor_tensor(a1[:], u[:], xt[:], ALU.add)
                            nc.sync.dma_start(adn1d_c[m], a1[:])
                            abf = abf_pool.tile([P, E], BF16, tag="abf",
                                                name=f"r{rep}_abf_{m}")
                            nc.scalar.copy(abf[:], a1[:])
                            for jj in range(NE):
                                tp = ps_t.tile([P, P], BF16, tag="tp",
                                               name=f"r{rep}_tpa_{m}_{jj}")
                                nc.tensor.transpose(tp[:],
                                                    abf[:, jj * P:(jj + 1) * P],
                                                    ident[:])
                                nc.scalar.copy(adn1T[jj][:, m * P:(m + 1) * P],
                                               tp[:])

                # -------------------------------------------- phase 3/4: FFN
            else:
                for jj in range(NE):
                    nc.sync.dma_start(adn1T[jj][:], xT_c[jj])
            if mode != "att":
                with ExitStack() as ffn_scope:
                    hT_pool = ffn_scope.enter_context(
                        tc.tile_pool(name="hTp", bufs=NF))
                    ps_f = ffn_scope.enter_context(
                        tc.tile_pool(name="ps_f", bufs=3, space="PSUM"))

                    with ExitStack() as ph3:
                        w1_pool = ph3.enter_context(tc.tile_pool(name="w1p", bufs=NE))
                        w1_sb = []
                        for c in range(NE):
                            t = w1_pool.tile([P, F], BF16, tag="w1", name=f"r{rep}_w1_{c}")
                            nc.sync.dma_start(t[:], w1_c[c])
                            w1_sb.append(t)
                        hT_sb = []
                        for j2 in range(NF):
                            ps = ps_f.tile([P, 1024], F32, tag="f", name=f"r{rep}_psf_{j2}")
                            for c in range(NE):
                                for n in range(2):
                                    nc.tensor.matmul(
                                        ps[:, n * 512:(n + 1) * 512],
                                        w1_sb[c][:, j2 * P:(j2 + 1) * P],
                                        adn1T[c][:, n * 512:(n + 1) * 512],
                                        start=(c == 0), stop=(c == NE - 1))
                            ht = hT_pool.tile([P, S], BF16, tag="hT",
                                              name=f"r{rep}_hT_{j2}")
                            nc.scalar.activation(ht[:], ps[:], AF.Gelu,
                                                 bias=b1_sb[:, j2:j2 + 1])
                            hT_sb.append(ht)

                    with ExitStack() as ph4:
                        w2_pool = ph4.enter_context(tc.tile_pool(name="w2p", bufs=NF))
                        gb2_pool = ph4.enter_context(tc.tile_pool(name="gb2p",
                                                                  bufs=1))
                        a1r_pool = ph4.enter_context(tc.tile_pool(name="a1rp",
                                                                  bufs=3))
                        ffb_pool = ph4.enter_context(tc.tile_pool(name="ffbp",
                                                                  bufs=2))
                        u2_pool = ph4.enter_context(tc.tile_pool(name="u2p", bufs=3))
                        st2_pool = ph4.enter_context(tc.tile_pool(name="st2p",
                                                                  bufs=4))
                        out_pool = ph4.enter_context(tc.tile_pool(name="outp",
                                                                  bufs=2))

                        b2_b = gb2_pool.tile([P, E], F32, tag="b2b")
                        gff_b = gb2_pool.tile([P, E], F32, tag="gffb")
                        bff_b = gb2_pool.tile([P, E], F32, tag="bffb")
                        g2_b = gb2_pool.tile([P, E], F32, tag="g2b")
                        be2_b = gb2_pool.tile([P, E], F32, tag="be2b")
                        nc.gpsimd.dma_start(b2_b[:], _bcast_ap(b2, E))
                        nc.gpsimd.dma_start(gff_b[:], _bcast_ap(gff, E))
                        nc.gpsimd.dma_start(bff_b[:], _bcast_ap(bff, E))
                        nc.gpsimd.dma_start(g2_b[:], _bcast_ap(g2, E))
                        nc.gpsimd.dma_start(be2_b[:], _bcast_ap(be2, E))

                        w2_sb = []
                        for c in range(NF):
                            t = w2_pool.tile([P, E], BF16, tag="w2", name=f"r{rep}_w2_{c}")
                            nc.sync.dma_start(t[:], w2_c[c])
                            w2_sb.append(t)

                        def ln_tile(src_ap, dst_ap, g_b, b_b, nm):
                            stats = st2_pool.tile([P, 2, 6], F32, tag="st2",
                                                  name=f"r{rep}_st2_{nm}")
                            mv = st2_pool.tile([P, 2], F32, tag="mv2",
                                               name=f"r{rep}_mv2_{nm}")
                            nc.vector.bn_stats(stats[:, 0, :], src_ap[:, 0:512])
                            nc.vector.bn_stats(stats[:, 1, :], src_ap[:, 512:1024])
                            nc.vector.bn_aggr(mv[:], stats[:])
                            rstd = st2_pool.tile([P, 1], F32, tag="rstd2",
                                                 name=f"r{rep}_rstd2_{nm}")
                            nc.scalar.activation(rstd[:], mv[:, 1:2], AF.Sqrt,
                                                 bias=eps_t[:])
                            nc.vector.reciprocal(rstd[:], rstd[:])
                            nc.vector.tensor_scalar(dst_ap, src_ap, mv[:, 0:1],
                                                    rstd[:], ALU.subtract, ALU.mult)
                            nc.vector.tensor_tensor(dst_ap, dst_ap, g_b[:], ALU.mult)
                            nc.vector.tensor_tensor(dst_ap, dst_ap, b_b[:], ALU.add)

                        for m in range(NS):
                            a1r = a1r_pool.tile([P, E], F32, tag="a1r",
                                                name=f"r{rep}_a1r_{m}")
                            nc.sync.dma_start(a1r[:], adn1d_c[m] if mode == "full" else x_c[m])
                            ps = ps_f.tile([P, 1024], F32, tag="f", name=f"r{rep}_ps2_{m}")
                            for c in range(NF):
                                for n in range(2):
                                    nc.tensor.matmul(
                                        ps[:, n * 512:(n + 1) * 512],
                                        hT_sb[c][:, m * P:(m + 1) * P],
                                        w2_sb[c][:, n * 512:(n + 1) * 512],
                                        start=(c == 0), stop=(c == NF - 1))
                            ffb = ffb_pool.tile([P, E], F32, tag="ffb",
                                                name=f"r{rep}_ffb_{m}")
                            nc.vector.tensor_tensor(ffb[:], ps[:], b2_b[:], ALU.add)
                            ln_tile(ffb[:], ffb[:], gff_b, bff_b, f"ff_{m}")
                            u2 = u2_pool.tile([P, E], F32, tag="u2", name=f"r{rep}_u2_{m}")
                            ln_tile(ffb[:], u2[:], g2_b, be2_b, f"l2_{m}")
                            ot = out_pool.tile([P, E], F32, tag="ot", name=f"r{rep}_ot_{m}")
                            nc.vector.tensor_tensor(ot[:], u2[:], a1r[:], ALU.add)
                            nc.sync.dma_start(out_c[m], ot[:])

    _split_sync_waits(nc)
    nc.finalize()
    return nc


_NC = {}


def _get_nc(reps=1, mode="full"):
    key = (reps, mode)
    if key not in _NC:
        _NC[key] = build(reps, mode)
    return _NC[key]


def make_in_maps(inputs):
    bf = ml_dtypes.bfloat16
    x = np.ascontiguousarray(np.asarray(inputs["x"], dtype=np.float32))
    shared = {
        "wq": np.ascontiguousarray(np.asarray(inputs["Wq"], np.float32).astype(bf)),
        "wk": np.ascontiguousarray(np.asarray(inputs["Wk"], np.float32).astype(bf)),
        "wv": np.ascontiguousarray(np.asarray(inputs["Wv"], np.float32).astype(bf)),
        "w1": np.ascontiguousarray(np.asarray(inputs["W1"], np.float32).astype(bf)),
        "w2": np.ascontiguousarray(np.asarray(inputs["W2"], np.float32).astype(bf)),
        "bq": np.asarray(inputs["bq"], np.float32),
        "bk": np.asarray(inputs["bk"], np.float32),
        "bv": np.asarray(inputs["bv"], np.float32),
        "b1": np.asarray(inputs["b1"], np.float32),
        "b2": np.asarray(inputs["b2"], np.float32),
        "g1": np.asarray(inputs["ln1_g"], np.float32),
        "be1": np.asarray(inputs["ln1_b"], np.float32),
        "gff": np.asarray(inputs["ln_ff_g"], np.float32),
        "bff": np.asarray(inputs["ln_ff_b"], np.float32),
        "g2": np.asarray(inputs["ln2_g"], np.float32),
        "be2": np.asarray(inputs["ln2_b"], np.float32),
    }
    in_maps = []
    for i in range(B):
        m = dict(shared)
        m["x"] = np.ascontiguousarray(x[i])
        m["xT"] = np.ascontiguousarray(x[i].T.astype(bf))
        in_maps.append(m)
    return in_maps


def kernel(**inputs):
    nc = _get_nc()
    in_maps = make_in_maps(inputs)
    res = run_bass_kernel_spmd(nc, in_maps, list(range(B)))
    return np.stack([res.results[i]["out"] for i in range(B)], axis=0)



# revision 12
# speedup vs baseline: 1.2797x; 1.2797x over previous
"""Trainium2 Bass kernel for nn_Block_52527450030210 (dense transformer block).

B=8, S=1024, E=1024, H=16 heads (D=64), F=4096. Data-parallel: batch element i
runs on core i (no collectives). Matmuls in bf16 with fp32 PSUM accumulation;
LayerNorms/softmax in fp32. Softmax is over the QUERY axis (dim=-2), so scores
are computed transposed ([k, q] layout) making the softmax reduction a
free-axis reduction, and 1/Z folds into v (Z is per contraction-index k).

v2 schedule: q/k projections are software-pipelined into the attention
k-tile loop to keep the PE array dense (HAM stays warm), softmax exps are
[128,1024]-wide (half the ACT instruction overhead), all transposes go
through the DMA xbar instead of the PE+ACT path, LN1 statistics are
accumulated incrementally during attention, FFN weights prefetch on the
SWDGE queue in halves sized to fit SBUF, and FFN1/FFN2 form one continuous
PE stream. adn1 stays in SBUF (bf16) instead of round-tripping through DRAM.

Self-contained: hardcodes shapes, includes the walrus single-sync-wait
workaround (this container's walrus accepts only one sync-wait per
instruction; Tile emits several, so extra waits are hoisted onto same-engine
NoOps).
"""

import numpy as np
import ml_dtypes

import concourse.bass as bass
import concourse.mybir as mybir
import concourse.tile as tile
from concourse.bass_utils import run_bass_kernel_spmd
from concourse.vector_clock import ScopedClock
from contextlib import ExitStack

F32 = mybir.dt.float32
BF16 = mybir.dt.bfloat16
AF = mybir.ActivationFunctionType
ALU = mybir.AluOpType

B, S, E, H, D, F = 8, 1024, 1024, 16, 64, 4096
P = 128
NE = E // P   # 8 e-chunks
NS = S // P   # 8 s-tiles
NF = F // P   # 32 f-tiles
EPS = 1e-5

# ---------------------------------------------------------------- waitfix ---

_wf_counter = [0]


def _wait_nop(nc, engine, wait, debug):
    _wf_counter[0] += 1
    nop = mybir.InstNoOp(
        name=f"I-wsplit-{_wf_counter[0]}", ins=[], outs=[], debug=debug,
        bass_nofuse=True,
    )
    nop.engine = engine
    nop.sync_info = mybir.SyncInfo(on_wait=[wait], on_update=[])
    nc.register_instruction(nop, overwrite=True)
    return nop


def _split_sync_waits(nc):
    for _name, bb in nc.bb_map.items():
        if not hasattr(bb, "instructions"):
            bb = bb.bb
        il = bb.instructions
        changed = False
        new = []
        for inst in il:
            si = inst.sync_info
            if si is not None and si.on_wait and len(si.on_wait) > 1:
                waits = list(si.on_wait)
                for w in waits[:-1]:
                    new.append(_wait_nop(nc, inst.engine, w, inst.debug))
                si.on_wait = waits[-1:]
                changed = True
            new.append(inst)
        if changed:
            bb.instructions = new


def _patched_drain_and_barrier(self, tick_clock, wait_clock):
    nop0 = self.nc.sync.nop(nofuse=True, hint="tail_wait")
    wait_clock.add_sem_waits(nop0.ins, ScopedClock({None: tick_clock.global_clock}))
    si = nop0.ins.sync_info
    waits = list(si.on_wait) if si and si.on_wait else []
    if len(waits) > 1:
        si.on_wait = waits[:1]
        rest = waits[1:]
        while rest:
            nop = self.nc.sync.nop(nofuse=True, hint="tail_wait")
            nop.ins.sync_info = mybir.SyncInfo(on_wait=rest[:1], on_update=[])
            rest = rest[1:]
    self.nc.sync.drain()
    self.nc.all_engine_barrier()
    assert self.sems is not None
    popped = self.nc._tile_sem_poison_stack.pop()
    assert popped is self._sem_poison
    self.nc.clear_and_free_semaphores(list(self.sems.allocated().values()))
    self.nc.all_engine_barrier()


tile.TileContext._drain_and_barrier = _patched_drain_and_barrier

# ----------------------------------------------------------------- build -----


def _bcast_ap(dram_t, n):
    """AP that DMA-broadcasts a [n] DRAM vector to [128, n] (partition step 0)."""
    return bass.AP(tensor=dram_t, offset=0, ap=[[0, P], [1, n]])


def build(reps=1, mode="full"):
    nc = bass.Bass()

    xT = nc.dram_tensor("xT", [E, S], BF16, kind="ExternalInput")
    x_f = nc.dram_tensor("x", [S, E], F32, kind="ExternalInput")
    wq = nc.dram_tensor("wq", [E, E], BF16, kind="ExternalInput")
    wk = nc.dram_tensor("wk", [E, E], BF16, kind="ExternalInput")
    wv = nc.dram_tensor("wv", [E, E], BF16, kind="ExternalInput")
    w1 = nc.dram_tensor("w1", [E, F], BF16, kind="ExternalInput")
    w2 = nc.dram_tensor("w2", [F, E], BF16, kind="ExternalInput")
    bq = nc.dram_tensor("bq", [E], F32, kind="ExternalInput")
    bk = nc.dram_tensor("bk", [E], F32, kind="ExternalInput")
    bv = nc.dram_tensor("bv", [E], F32, kind="ExternalInput")
    b1 = nc.dram_tensor("b1", [F], F32, kind="ExternalInput")
    b2 = nc.dram_tensor("b2", [E], F32, kind="ExternalInput")
    g1 = nc.dram_tensor("g1", [E], F32, kind="ExternalInput")
    be1 = nc.dram_tensor("be1", [E], F32, kind="ExternalInput")
    gff = nc.dram_tensor("gff", [E], F32, kind="ExternalInput")
    bff = nc.dram_tensor("bff", [E], F32, kind="ExternalInput")
    g2 = nc.dram_tensor("g2", [E], F32, kind="ExternalInput")
    be2 = nc.dram_tensor("be2", [E], F32, kind="ExternalInput")

    xT_c = xT.rearrange("(c p) s -> c p s", p=P)
    x_c = x_f.rearrange("(m p) e -> m p e", p=P)
    wq_c = wq.rearrange("(c p) e -> c p e", p=P)
    wk_c = wk.rearrange("(c p) e -> c p e", p=P)
    wv_c = wv.rearrange("(c p) e -> c p e", p=P)
    w1_c = w1.rearrange("(c p) f -> c p f", p=P)
    w2_c = w2.rearrange("(c p) e -> c p e", p=P)

    with tile.TileContext(nc) as tc:
      for rep in range(reps):
        out_d = nc.dram_tensor("out" if rep == 0 else f"out_r{rep}",
                               [S, E], F32, kind="ExternalOutput")
        out_c = out_d.rearrange("(m p) e -> m p e", p=P)
        with ExitStack() as top:
            const = top.enter_context(tc.tile_pool(name="const", bufs=1))
            eps_t = const.tile([P, 1], F32)
            nc.vector.memset(eps_t[:], EPS)
            bq_sb = const.tile([P, NE], F32)
            bk_sb = const.tile([P, NE], F32)
            b1_sb = const.tile([P, NF], F32)
            nc.sync.dma_start(bq_sb[:], bq.rearrange("(c p) -> p c", p=P))
            nc.sync.dma_start(bk_sb[:], bk.rearrange("(c p) -> p c", p=P))
            nc.sync.dma_start(b1_sb[:], b1.rearrange("(c p) -> p c", p=P))

            # residual (bf16) and its transpose live across the whole rep
            a1_pool = top.enter_context(tc.tile_pool(name="a1p", bufs=NS))
            a1T_pool = top.enter_context(tc.tile_pool(name="a1Tp", bufs=2 * NE))
            a1bf = [a1_pool.tile([P, E], BF16, tag="a1", name=f"r{rep}_a1_{m}")
                    for m in range(NS)]
            # adn1T split by s-half so FFN1 n=0 only depends on LN1 of m=0..3
            a1T = [[a1T_pool.tile([P, S // 2], BF16, tag="a1T",
                                  name=f"r{rep}_a1T_{c}_{n}") for n in range(2)]
                   for c in range(NE)]

            # ---------------------------------------------- attention ----
            with ExitStack() as attn:
                att_pool = attn.enter_context(tc.tile_pool(name="attp",
                                                           bufs=NS))
                st_pool = attn.enter_context(tc.tile_pool(name="stp", bufs=NS))
                bc1_pool = attn.enter_context(tc.tile_pool(name="bc1p",
                                                           bufs=1))
                v_pool = attn.enter_context(tc.tile_pool(name="vp", bufs=NS))
                qk_pool = attn.enter_context(tc.tile_pool(name="qkp", bufs=6))
                exp_pool = attn.enter_context(tc.tile_pool(name="expp",
                                                           bufs=3))
                small_pool = attn.enter_context(tc.tile_pool(name="smallp",
                                                             bufs=8))
                attjT_pool = attn.enter_context(tc.tile_pool(name="attjTp",
                                                             bufs=1))
                u_pool = attn.enter_context(tc.tile_pool(name="up", bufs=1))

                att_sb = [att_pool.tile([P, E], BF16, tag="att",
                                        name=f"r{rep}_att_{m}")
                          for m in range(NS)]
                st1 = [st_pool.tile([P, 4, 6], F32, tag="st1",
                                    name=f"r{rep}_st1_{m}")
                       for m in range(NS)]

                bv_b = bc1_pool.tile([P, E], BF16, tag="bvb")
                g1_b = bc1_pool.tile([P, E], BF16, tag="g1b")
                be1_b = bc1_pool.tile([P, E], BF16, tag="be1b")
                nc.gpsimd.dma_start(bv_b[:], _bcast_ap(bv, E))
                nc.gpsimd.dma_start(g1_b[:], _bcast_ap(g1, E))
                nc.gpsimd.dma_start(be1_b[:], _bcast_ap(be1, E))

                scopeA = ExitStack()      # xT + wq/wk (freed after last qk)
                attn.enter_context(scopeA)
                xT_pool = scopeA.enter_context(tc.tile_pool(name="xTp",
                                                            bufs=NE))
                wqk_pool = scopeA.enter_context(tc.tile_pool(name="wqkp",
                                                             bufs=NE))

                # xT + wv first (v starts ASAP), then wq/wk
                xT_sb, wq_sb, wk_sb = [], [], []
                with ExitStack() as vscope:
                    wv_pool = vscope.enter_context(tc.tile_pool(name="wvp",
                                                                bufs=NE))
                    ps_v = vscope.enter_context(
                        tc.tile_pool(name="ps_v", bufs=2, space="PSUM"))
                    wv_sb = []
                    for c in range(NE):
                        t = xT_pool.tile([P, S], BF16, tag="xT",
                                         name=f"r{rep}_xT_{c}")
                        nc.sync.dma_start(t[:], xT_c[c])
                        xT_sb.append(t)
                        tv = wv_pool.tile([P, E], BF16, tag="wv",
                                          name=f"r{rep}_wv_{c}")
                        nc.sync.dma_start(tv[:], wv_c[c])
                        wv_sb.append(tv)
                    for c in range(NE):
                        tq = wqk_pool.tile([P, E], BF16, tag="wq",
                                           name=f"r{rep}_wq_{c}")
                        tk = wqk_pool.tile([P, E], BF16, tag="wk",
                                           name=f"r{rep}_wk_{c}")
                        nc.sync.dma_start(tq[:], wq_c[c])
                        nc.sync.dma_start(tk[:], wk_c[c])
                        wq_sb.append(tq)
                        wk_sb.append(tk)

                    # v[s, e] = x @ Wv + bv
                    v_sb = []
                    for m in range(NS):
                        vt = v_pool.tile([P, E], BF16, tag="v",
                                         name=f"r{rep}_v_{m}")
                        ps = ps_v.tile([P, 1024], F32, tag="vps",
                                       name=f"r{rep}_psv_{m}")
                        for n in range(2):
                            for c in range(NE):
                                nc.tensor.matmul(
                                    ps[:, n * 512:(n + 1) * 512],
                                    xT_sb[c][:, m * P:(m + 1) * P],
                                    wv_sb[c][:, n * 512:(n + 1) * 512],
                                    start=(c == 0), stop=(c == NE - 1))
                        nc.vector.tensor_tensor(vt[:], ps[:], bv_b[:], ALU.add)
                        v_sb.append(vt)

                # a1bf[m] = x[m] + be1 (bf16; SWDGE casts f32->bf16 in the DMA)
                for m in range(NS):
                    nc.gpsimd.dma_start(a1bf[m][:], x_c[m])
                    nc.vector.tensor_tensor(a1bf[m][:], a1bf[m][:], be1_b[:],
                                            ALU.add)

                ps_big = attn.enter_context(
                    tc.tile_pool(name="ps_big", bufs=2, space="PSUM"))
                ps_av = attn.enter_context(
                    tc.tile_pool(name="ps_av", bufs=2, space="PSUM"))

                def emit_qk(j):
                    """qT[j], kT[j] (bf16 [P,S]) via shared ps_big pool."""
                    qt = qk_pool.tile([P, S], BF16, tag="qkT",
                                      name=f"r{rep}_qT_{j}")
                    kt = qk_pool.tile([P, S], BF16, tag="qkT",
                                      name=f"r{rep}_kT_{j}")
                    for di, (dst, w_sb, bias) in enumerate(
                            ((qt, wq_sb, bq_sb), (kt, wk_sb, bk_sb))):
                        ps = ps_big.tile([P, 1024], F32, tag="big",
                                         name=f"r{rep}_psqk_{j}_{di}")
                        for n in range(2):
                            for c in range(NE):
                                nc.tensor.matmul(
                                    ps[:, n * 512:(n + 1) * 512],
                                    w_sb[c][:, j * P:(j + 1) * P],
                                    xT_sb[c][:, n * 512:(n + 1) * 512],
                                    start=(c == 0), stop=(c == NE - 1))
                        nc.vector.tensor_scalar_add(dst[:], ps[:],
                                                    bias[:, j:j + 1])
                    return qt, kt

                qk_sb = {0: emit_qk(0), 1: emit_qk(1)}

                for j in range(NE):
                    qt, kt = qk_sb.pop(j)
                    avp = ps_av.tile([P, 1024], F32, tag="av",
                                     name=f"r{rep}_av_{j}")
                    for kc in range(NS):
                        for h in range(2):
                            o = 64 * h
                            scps = ps_big.tile([P, 1024], F32, tag="big",
                                               name=f"r{rep}_sc_{j}_{kc}_{h}")
                            for n in range(2):
                                nc.tensor.matmul(
                                    scps[:, n * 512:(n + 1) * 512],
                                    kt[o:o + 64, kc * P:(kc + 1) * P],
                                    qt[o:o + 64, n * 512:(n + 1) * 512],
                                    start=True, stop=True,
                                    tile_position=(o, 0))
                            expt = exp_pool.tile([P, 1024], BF16, tag="expt",
                                                 name=f"r{rep}_ex_{j}_{kc}_{h}")
                            zt = small_pool.tile([P, 1], F32, tag="z",
                                                 name=f"r{rep}_z_{j}_{kc}_{h}")
                            nc.scalar.activation(expt[:], scps[:], AF.Exp,
                                                 scale=0.1, accum_out=zt[:])
                            nc.vector.reciprocal(zt[:], zt[:])
                            vsc = small_pool.tile([P, 64], BF16, tag="vsc",
                                                  name=f"r{rep}_vs_{j}_{kc}_{h}")
                            head = 2 * j + h
                            nc.vector.tensor_scalar_mul(
                                vsc[:], v_sb[kc][:, head * 64:(head + 1) * 64],
                                zt[:])
                            for n in range(2):
                                nc.tensor.matmul(
                                    avp[o:o + 64, n * 512:(n + 1) * 512],
                                    vsc[:], expt[:, n * 512:(n + 1) * 512],
                                    start=(kc == 0), stop=(kc == NS - 1),
                                    tile_position=(0, o))
                        # interleave next-j qk matmuls into the kc loop
                        if kc == 3 and j + 2 < NE:
                            qk_sb[j + 2] = emit_qk(j + 2)

                    attjT = attjT_pool.tile([P, S], BF16, tag="attjT",
                                            name=f"r{rep}_attjT_{j}")
                    nc.vector.tensor_copy(attjT[:], avp[:])
                    for m in range(NS):
                        nc.sync.dma_start_transpose(
                            att_sb[m][:, j * P:(j + 1) * P],
                            attjT[:, m * P:(m + 1) * P])
                    if j % 2 == 1:
                        for m in range(NS):
                            nc.vector.bn_stats(
                                st1[m][:, j // 2, :],
                                att_sb[m][:, (j - 1) * P:(j + 1) * P])
                    if j == NE - 2:
                        scopeA.close()   # free xT/wq/wk for w1 prefetch

                # -------------------------------------- LN1 + adn1T ----
                mv1, rstd1 = [], []
                for m in range(NS):
                    mv = st_pool.tile([P, 2], F32, tag="mv1",
                                      name=f"r{rep}_mv1_{m}")
                    nc.vector.bn_aggr(mv[:], st1[m][:])
                    rstd = st_pool.tile([P, 1], F32, tag="rstd1",
                                        name=f"r{rep}_rstd1_{m}")
                    nc.scalar.activation(rstd[:], mv[:, 1:2], AF.Sqrt,
                                         bias=eps_t[:])
                    nc.vector.reciprocal(rstd[:], rstd[:])
                    mv1.append(mv)
                    rstd1.append(rstd)
                for m in range(NS):
                    u = u_pool.tile([P, E], F32, tag="u", name=f"r{rep}_u_{m}")
                    nc.vector.tensor_scalar(u[:], att_sb[m][:], mv1[m][:, 0:1],
                                            rstd1[m][:], ALU.subtract, ALU.mult)
                    nc.vector.tensor_tensor(u[:], u[:], g1_b[:], ALU.mult)
                    nc.vector.tensor_tensor(a1bf[m][:], a1bf[m][:], u[:],
                                            ALU.add)
                    for c in range(NE):
                        nc.sync.dma_start_transpose(
                            a1T[c][m // 4][:, (m % 4) * P:(m % 4 + 1) * P],
                            a1bf[m][:, c * P:(c + 1) * P])

            # ------------------------------------------------ FFN ----
            with ExitStack() as ffn:
                hT_pool = ffn.enter_context(tc.tile_pool(name="hTp",
                                                         bufs=2 * NF))
                ps_f = ffn.enter_context(
                    tc.tile_pool(name="ps_f", bufs=4, space="PSUM"))
                w2a_pool = ffn.enter_context(tc.tile_pool(name="w2ap",
                                                          bufs=NF // 2))
                scopeW1 = ExitStack()     # w1 halves; closed after FFN1
                ffn.enter_context(scopeW1)
                w1a_pool = scopeW1.enter_context(
                    tc.tile_pool(name="w1ap", bufs=NE))
                w1b_pool = scopeW1.enter_context(
                    tc.tile_pool(name="w1bp", bufs=NE))
                w1a_sb, w1b_sb, w2_sb = [], [], []
                for c in range(NE):
                    t = w1a_pool.tile([P, F // 2], BF16, tag="w1a",
                                      name=f"r{rep}_w1a_{c}")
                    nc.gpsimd.dma_start(t[:], w1_c[c][:, 0:F // 2])
                    w1a_sb.append(t)
                for c in range(NE):
                    t = w1b_pool.tile([P, F // 2], BF16, tag="w1b",
                                      name=f"r{rep}_w1b_{c}")
                    nc.gpsimd.dma_start(t[:], w1_c[c][:, F // 2:F])
                    w1b_sb.append(t)
                for c in range(NF // 2):
                    t = w2a_pool.tile([P, E], BF16, tag="w2a",
                                      name=f"r{rep}_w2_{c}")
                    nc.gpsimd.dma_start(t[:], w2_c[c])
                    w2_sb.append(t)

                # hT split by s-half (FFN2 m<4 only needs the n=0 half)
                hT = [[hT_pool.tile([P, S // 2], BF16, tag="hT",
                                    name=f"r{rep}_hT_{f}_{n}") for n in range(2)]
                      for f in range(NF)]

                for n in range(2):
                    for f in range(NF):
                        ps = ps_f.tile([P, 512], F32, tag="f",
                                       name=f"r{rep}_psf_{f}_{n}")
                        w_half = w1a_sb if f < NF // 2 else w1b_sb
                        fo = f if f < NF // 2 else f - NF // 2
                        for c in range(NE):
                            nc.tensor.matmul(
                                ps[:],
                                w_half[c][:, fo * P:(fo + 1) * P],
                                a1T[c][n][:],
                                start=(c == 0), stop=(c == NE - 1))
                        nc.scalar.activation(hT[f][n][:], ps[:], AF.Gelu,
                                             bias=b1_sb[:, f:f + 1])

                scopeW1.close()   # free w1 halves; a1T no longer needed
                with ExitStack() as ffn2:
                    ps_2 = ffn2.enter_context(
                        tc.tile_pool(name="ps_2", bufs=2, space="PSUM"))
                    w2b_pool = ffn2.enter_context(tc.tile_pool(name="w2bp",
                                                               bufs=NF // 2))
                    bc2_pool = ffn2.enter_context(tc.tile_pool(name="bc2p",
                                                               bufs=1))
                    ff_pool = ffn2.enter_context(tc.tile_pool(name="ffp",
                                                              bufs=2))
                    u2_pool = ffn2.enter_context(tc.tile_pool(name="u2p",
                                                              bufs=2))
                    st2_pool = ffn2.enter_context(tc.tile_pool(name="st2p",
                                                               bufs=6))
                    out_pool = ffn2.enter_context(tc.tile_pool(name="outp",
                                                               bufs=2))
                    for c in range(NF // 2, NF):
                        t = w2b_pool.tile([P, E], BF16, tag="w2b",
                                          name=f"r{rep}_w2_{c}")
                        nc.gpsimd.dma_start(t[:], w2_c[c])
                        w2_sb.append(t)

                    b2_b = bc2_pool.tile([P, E], F32, tag="b2b")
                    gff_b = bc2_pool.tile([P, E], F32, tag="gffb")
                    bff_b = bc2_pool.tile([P, E], F32, tag="bffb")
                    g2_b = bc2_pool.tile([P, E], F32, tag="g2b")
                    be2_b = bc2_pool.tile([P, E], F32, tag="be2b")
                    nc.gpsimd.dma_start(b2_b[:], _bcast_ap(b2, E))
                    nc.gpsimd.dma_start(gff_b[:], _bcast_ap(gff, E))
                    nc.gpsimd.dma_start(bff_b[:], _bcast_ap(bff, E))
                    nc.gpsimd.dma_start(g2_b[:], _bcast_ap(g2, E))
                    nc.gpsimd.dma_start(be2_b[:], _bcast_ap(be2, E))

                    def ln_tile(src_ap, dst_ap, g_b, b_b, nm):
                        stats = st2_pool.tile([P, 2, 6], F32, tag="st2",
                                              name=f"r{rep}_st2_{nm}")
                        mv = st2_pool.tile([P, 2], F32, tag="mv2",
                                           name=f"r{rep}_mv2_{nm}")
                        nc.vector.bn_stats(stats[:, 0, :], src_ap[:, 0:512])
                        nc.vector.bn_stats(stats[:, 1, :], src_ap[:, 512:1024])
                        nc.vector.bn_aggr(mv[:], stats[:])
                        rstd = st2_pool.tile([P, 1], F32, tag="rstd2",
                                             name=f"r{rep}_rstd2_{nm}")
                        nc.scalar.activation(rstd[:], mv[:, 1:2], AF.Sqrt,
                                             bias=eps_t[:])
                        nc.vector.reciprocal(rstd[:], rstd[:])
                        nc.vector.tensor_scalar(dst_ap, src_ap, mv[:, 0:1],
                                                rstd[:], ALU.subtract,
                                                ALU.mult)
                        nc.vector.tensor_tensor(dst_ap, dst_ap, g_b[:],
                                                ALU.mult)
                        nc.vector.tensor_tensor(dst_ap, dst_ap, b_b[:],
                                                ALU.add)

                    for m in range(NS):
                        ps = ps_2.tile([P, 1024], F32, tag="o",
                                       name=f"r{rep}_ps2_{m}")
                        half, mm = (0, m) if m < 4 else (1, m - 4)
                        for c in range(NF):
                            for n in range(2):
                                nc.tensor.matmul(
                                    ps[:, n * 512:(n + 1) * 512],
                                    hT[c][half][:, mm * P:(mm + 1) * P],
                                    w2_sb[c][:, n * 512:(n + 1) * 512],
                                    start=(c == 0), stop=(c == NF - 1))
                        ffb = ff_pool.tile([P, E], F32, tag="ffb",
                                           name=f"r{rep}_ffb_{m}")
                        nc.vector.tensor_tensor(ffb[:], ps[:], b2_b[:],
                                                ALU.add)
                        ln_tile(ffb[:], ffb[:], gff_b, bff_b, f"ff_{m}")
                        u2 = u2_pool.tile([P, E], F32, tag="u2",
                                          name=f"r{rep}_u2_{m}")
                        ln_tile(ffb[:], u2[:], g2_b, be2_b, f"l2_{m}")
                        ot = out_pool.tile([P, E], F32, tag="ot",
                                           name=f"r{rep}_ot_{m}")
                        nc.vector.tensor_tensor(ot[:], u2[:], a1bf[m][:],
                                                ALU.add)
                        nc.sync.dma_start(out_c[m], ot[:])

    _split_sync_waits(nc)
    nc.finalize()
    return nc


_NC = {}


def _get_nc(reps=1, mode="full"):
    key = (reps, mode)
    if key not in _NC:
        _NC[key] = build(reps, mode)
    return _NC[key]


def make_in_maps(inputs):
    bf = ml_dtypes.bfloat16
    x = np.ascontiguousarray(np.asarray(inputs["x"], dtype=np.float32))
    shared = {
        "wq": np.ascontiguousarray(np.asarray(inputs["Wq"], np.float32).astype(bf)),
        "wk": np.ascontiguousarray(np.asarray(inputs["Wk"], np.float32).astype(bf)),
        "wv": np.ascontiguousarray(np.asarray(inputs["Wv"], np.float32).astype(bf)),
        "w1": np.ascontiguousarray(np.asarray(inputs["W1"], np.float32).astype(bf)),
        "w2": np.ascontiguousarray(np.asarray(inputs["W2"], np.float32).astype(bf)),
        "bq": np.asarray(inputs["bq"], np.float32),
        "bk": np.asarray(inputs["bk"], np.float32),
        "bv": np.asarray(inputs["bv"], np.float32),
        "b1": np.asarray(inputs["b1"], np.float32),
        "b2": np.asarray(inputs["b2"], np.float32),
        "g1": np.asarray(inputs["ln1_g"], np.float32),
        "be1": np.asarray(inputs["ln1_b"], np.float32),
        "gff": np.asarray(inputs["ln_ff_g"], np.float32),
        "bff": np.asarray(inputs["ln_ff_b"], np.float32),
        "g2": np.asarray(inputs["ln2_g"], np.float32),
        "be2": np.asarray(inputs["ln2_b"], np.float32),
    }
    in_maps = []
    for i in range(B):
        m = dict(shared)
        m["x"] = np.ascontiguousarray(x[i])
        m["xT"] = np.ascontiguousarray(x[i].T.astype(bf))
        in_maps.append(m)
    return in_maps


def kernel(**inputs):
    nc = _get_nc()
    in_maps = make_in_maps(inputs)
    res = run_bass_kernel_spmd(nc, in_maps, list(range(B)))
    return np.stack([res.results[i]["out"] for i in range(B)], axis=0)


# revision 25
# speedup vs baseline: 1.3090x; 1.0229x over previous
"""Trainium2 Bass kernel for nn_Block_52527450030210 (dense transformer block).

B=8, S=1024, E=1024, H=16 heads (D=64), F=4096. Data-parallel: batch element i
runs on core i (no collectives). Matmuls in bf16 with fp32 PSUM accumulation;
LayerNorms/softmax in fp32. Softmax is over the QUERY axis (dim=-2), so scores
are computed transposed ([k, q] layout) making the softmax reduction a
free-axis reduction, and 1/Z folds into v (Z is per contraction-index k).

v2 schedule: q/k projections are software-pipelined into the attention
k-tile loop to keep the PE array dense (HAM stays warm), softmax exps are
[128,1024]-wide (half the ACT instruction overhead), all transposes go
through the DMA xbar instead of the PE+ACT path, LN1 statistics are
accumulated incrementally during attention, FFN weights prefetch on the
SWDGE queue in halves sized to fit SBUF, and FFN1/FFN2 form one continuous
PE stream. adn1 stays in SBUF (bf16) instead of round-tripping through DRAM.

Self-contained: hardcodes shapes, includes the walrus single-sync-wait
workaround (this container's walrus accepts only one sync-wait per
instruction; Tile emits several, so extra waits are hoisted onto same-engine
NoOps).
"""

import numpy as np
import ml_dtypes

import concourse.bass as bass
import concourse.mybir as mybir
import concourse.tile as tile
from concourse.bass_utils import run_bass_kernel_spmd
from concourse.vector_clock import ScopedClock
from contextlib import ExitStack

F32 = mybir.dt.float32
BF16 = mybir.dt.bfloat16
AF = mybir.ActivationFunctionType
ALU = mybir.AluOpType

B, S, E, H, D, F = 8, 1024, 1024, 16, 64, 4096
P = 128
NE = E // P   # 8 e-chunks
NS = S // P   # 8 s-tiles
NF = F // P   # 32 f-tiles
EPS = 1e-5

# ---------------------------------------------------------------- waitfix ---

_wf_counter = [0]


def _wait_nop(nc, engine, wait, debug):
    _wf_counter[0] += 1
    nop = mybir.InstNoOp(
        name=f"I-wsplit-{_wf_counter[0]}", ins=[], outs=[], debug=debug,
        bass_nofuse=True,
    )
    nop.engine = engine
    nop.sync_info = mybir.SyncInfo(on_wait=[wait], on_update=[])
    nc.register_instruction(nop, overwrite=True)
    return nop


def _split_sync_waits(nc):
    for _name, bb in nc.bb_map.items():
        if not hasattr(bb, "instructions"):
            bb = bb.bb
        il = bb.instructions
        changed = False
        new = []
        for inst in il:
            si = inst.sync_info
            if si is not None and si.on_wait and len(si.on_wait) > 1:
                waits = list(si.on_wait)
                for w in waits[:-1]:
                    new.append(_wait_nop(nc, inst.engine, w, inst.debug))
                si.on_wait = waits[-1:]
                changed = True
            new.append(inst)
        if changed:
            bb.instructions = new


def _patched_drain_and_barrier(self, tick_clock, wait_clock):
    nop0 = self.nc.sync.nop(nofuse=True, hint="tail_wait")
    wait_clock.add_sem_waits(nop0.ins, ScopedClock({None: tick_clock.global_clock}))
    si = nop0.ins.sync_info
    waits = list(si.on_wait) if si and si.on_wait else []
    if len(waits) > 1:
        si.on_wait = waits[:1]
        rest = waits[1:]
        while rest:
            nop = self.nc.sync.nop(nofuse=True, hint="tail_wait")
            nop.ins.sync_info = mybir.SyncInfo(on_wait=rest[:1], on_update=[])
            rest = rest[1:]
    self.nc.sync.drain()
    self.nc.all_engine_barrier()
    assert self.sems is not None
    popped = self.nc._tile_sem_poison_stack.pop()
    assert popped is self._sem_poison
    self.nc.clear_and_free_semaphores(list(self.sems.allocated().values()))
    self.nc.all_engine_barrier()


tile.TileContext._drain_and_barrier = _patched_drain_and_barrier

# ----------------------------------------------------------------- build -----


def _bcast_ap(dram_t, n):
    """AP that DMA-broadcasts a [n] DRAM vector to [128, n] (partition step 0)."""
    return bass.AP(tensor=dram_t, offset=0, ap=[[0, P], [1, n]])


def build(reps=1, mode="full"):
    nc = bass.Bass()

    xT = nc.dram_tensor("xT", [E, S], BF16, kind="ExternalInput")
    x_f = nc.dram_tensor("x", [S, E], F32, kind="ExternalInput")
    wq = nc.dram_tensor("wq", [E, E], BF16, kind="ExternalInput")
    wk = nc.dram_tensor("wk", [E, E], BF16, kind="ExternalInput")
    wv = nc.dram_tensor("wv", [E, E], BF16, kind="ExternalInput")
    w1 = nc.dram_tensor("w1", [E, F], BF16, kind="ExternalInput")
    w2 = nc.dram_tensor("w2", [F, E], BF16, kind="ExternalInput")
    bq = nc.dram_tensor("bq", [E], F32, kind="ExternalInput")
    bk = nc.dram_tensor("bk", [E], F32, kind="ExternalInput")
    bv = nc.dram_tensor("bv", [E], F32, kind="ExternalInput")
    b1 = nc.dram_tensor("b1", [F], F32, kind="ExternalInput")
    b2 = nc.dram_tensor("b2", [E], F32, kind="ExternalInput")
    g1 = nc.dram_tensor("g1", [E], F32, kind="ExternalInput")
    be1 = nc.dram_tensor("be1", [E], F32, kind="ExternalInput")
    gff = nc.dram_tensor("gff", [E], F32, kind="ExternalInput")
    bff = nc.dram_tensor("bff", [E], F32, kind="ExternalInput")
    g2 = nc.dram_tensor("g2", [E], F32, kind="ExternalInput")
    be2 = nc.dram_tensor("be2", [E], F32, kind="ExternalInput")

    xT_c = xT.rearrange("(c p) s -> c p s", p=P)
    x_c = x_f.rearrange("(m p) e -> m p e", p=P)
    wq_c = wq.rearrange("(c p) e -> c p e", p=P)
    wk_c = wk.rearrange("(c p) e -> c p e", p=P)
    wv_c = wv.rearrange("(c p) e -> c p e", p=P)
    w1_c = w1.rearrange("(c p) f -> c p f", p=P)
    w2_c = w2.rearrange("(c p) e -> c p e", p=P)

    with tile.TileContext(nc) as tc:
      for rep in range(reps):
        out_d = nc.dram_tensor("out" if rep == 0 else f"out_r{rep}",
                               [S, E], F32, kind="ExternalOutput")
        out_c = out_d.rearrange("(m p) e -> m p e", p=P)
        with ExitStack() as top:
            const = top.enter_context(tc.tile_pool(name="const", bufs=1))
            eps_t = const.tile([P, 1], F32)
            nc.vector.memset(eps_t[:], EPS)
            bq_sb = const.tile([P, NE], F32)
            bk_sb = const.tile([P, NE], F32)
            b1_sb = const.tile([P, NF], F32)
            nc.sync.dma_start(bq_sb[:], bq.rearrange("(c p) -> p c", p=P))
            nc.sync.dma_start(bk_sb[:], bk.rearrange("(c p) -> p c", p=P))
            nc.sync.dma_start(b1_sb[:], b1.rearrange("(c p) -> p c", p=P))

            # residual (bf16) and its transpose live across the whole rep
            a1_pool = top.enter_context(tc.tile_pool(name="a1p", bufs=NS))
            a1T_pool = top.enter_context(tc.tile_pool(name="a1Tp", bufs=2))
            a1bf = [a1_pool.tile([P, E], BF16, tag="a1", name=f"r{rep}_a1_{m}")
                    for m in range(NS)]
            # adn1T split by s-half so FFN1 n=0 only depends on LN1 of m=0..3;
            # [P, c, s] group-fold layout filled by one xbar transpose per m
            a1T = [a1T_pool.tile([P, NE, S // 2], BF16, tag="a1T",
                                 name=f"r{rep}_a1T_{n}") for n in range(2)]

            # ---------------------------------------------- attention ----
            with ExitStack() as attn:
                att_pool = attn.enter_context(tc.tile_pool(name="attp",
                                                           bufs=1))
                st_pool = attn.enter_context(tc.tile_pool(name="stp", bufs=NS))
                bc1_pool = attn.enter_context(tc.tile_pool(name="bc1p",
                                                           bufs=1))
                v_pool = attn.enter_context(tc.tile_pool(name="vp", bufs=NS))
                qk_pool = attn.enter_context(tc.tile_pool(name="qkp", bufs=6))
                exp_pool = attn.enter_context(tc.tile_pool(name="expp",
                                                           bufs=6))
                small_pool = attn.enter_context(tc.tile_pool(name="smallp",
                                                             bufs=8))
                wqkj_pool = attn.enter_context(tc.tile_pool(name="wqkjp",
                                                            bufs=6))
                attjT_pool = attn.enter_context(tc.tile_pool(name="attjTp",
                                                             bufs=1))
                u_pool = attn.enter_context(tc.tile_pool(name="up", bufs=1))

                # att in [s, e] layout: one tile, group-fold m = dim 1
                att_all = att_pool.tile([P, NS, E], BF16,
                                        name=f"r{rep}_att_all")
                st1 = [st_pool.tile([P, 4, 6], F32, tag="st1",
                                    name=f"r{rep}_st1_{m}")
                       for m in range(NS)]

                bv_b = bc1_pool.tile([P, E], BF16, tag="bvb")
                g1_b = bc1_pool.tile([P, E], BF16, tag="g1b")
                be1_b = bc1_pool.tile([P, E], BF16, tag="be1b")
                nc.gpsimd.dma_start(bv_b[:], _bcast_ap(bv, E))
                nc.gpsimd.dma_start(g1_b[:], _bcast_ap(g1, E))
                nc.gpsimd.dma_start(be1_b[:], _bcast_ap(be1, E))

                scopeA = ExitStack()      # xT (freed after last qk)
                attn.enter_context(scopeA)
                xT_pool = scopeA.enter_context(tc.tile_pool(name="xTp",
                                                            bufs=NE))

                # per-j [P, c, 128] slices of wq/wk; full tensors stay in DRAM
                wq_j = wq.rearrange("(c p) e -> p c e", p=P)
                wk_j = wk.rearrange("(c p) e -> p c e", p=P)

                # xT + wv first (v starts ASAP)
                xT_sb = []
                wv_scope = ExitStack()
                attn.enter_context(wv_scope)
                wv_pool = wv_scope.enter_context(tc.tile_pool(name="wvp",
                                                              bufs=NE))
                wv_sb = []
                for c in range(NE):
                    t = xT_pool.tile([P, S], BF16, tag="xT",
                                     name=f"r{rep}_xT_{c}")
                    nc.sync.dma_start(t[:], xT_c[c])
                    xT_sb.append(t)
                    tv = wv_pool.tile([P, E], BF16, tag="wv",
                                      name=f"r{rep}_wv_{c}")
                    nc.sync.dma_start(tv[:], wv_c[c])
                    wv_sb.append(tv)
                for m in range(NS):
                    nc.gpsimd.dma_start(a1bf[m][:], x_c[m])

                ps_big = attn.enter_context(
                    tc.tile_pool(name="ps_big", bufs=3, space="PSUM"))
                ps_av = attn.enter_context(
                    tc.tile_pool(name="ps_av", bufs=1, space="PSUM"))

                def emit_qk(j):
                    """qT[j], kT[j] (bf16 [P,S]) via shared ps_big pool."""
                    qt = qk_pool.tile([P, S], BF16, tag="qkT",
                                      name=f"r{rep}_qT_{j}")
                    kt = qk_pool.tile([P, S], BF16, tag="qkT",
                                      name=f"r{rep}_kT_{j}")
                    for di, (dst, w_dram, bias) in enumerate(
                            ((qt, wq_j, bq_sb), (kt, wk_j, bk_sb))):
                        wj = wqkj_pool.tile([P, NE, P], BF16, tag="wqkj",
                                            name=f"r{rep}_wj_{j}_{di}")
                        nc.sync.dma_start(wj[:],
                                          w_dram[:, :, j * P:(j + 1) * P])
                        ps = ps_big.tile([P, 1024], F32, tag="big",
                                         name=f"r{rep}_psqk_{j}_{di}")
                        for n in range(2):
                            for c in range(NE):
                                nc.tensor.matmul(
                                    ps[:, n * 512:(n + 1) * 512],
                                    wj[:, c, :],
                                    xT_sb[c][:, n * 512:(n + 1) * 512],
                                    start=(c == 0), stop=(c == NE - 1))
                        nc.vector.tensor_scalar_add(dst[:], ps[:],
                                                    bias[:, j:j + 1])
                    return qt, kt

                def emit_v(m):
                    vt = v_pool.tile([P, E], BF16, tag="v", name=f"r{rep}_v_{m}")
                    ps = ps_big.tile([P, 1024], F32, tag="big",
                                     name=f"r{rep}_psv_{m}")
                    for n in range(2):
                        for c in range(NE):
                            nc.tensor.matmul(
                                ps[:, n * 512:(n + 1) * 512],
                                xT_sb[c][:, m * P:(m + 1) * P],
                                wv_sb[c][:, n * 512:(n + 1) * 512],
                                start=(c == 0), stop=(c == NE - 1))
                    nc.vector.tensor_tensor(vt[:], ps[:], bv_b[:], ALU.add)
                    return vt

                v_sb = [emit_v(m) for m in range(3)]
                qk_sb = {0: emit_qk(0)}

                def emit_sc_exp(j, kc, h, qt, kt):
                    o = 64 * h
                    scps = ps_big.tile([P, 1024], F32, tag="big",
                                       name=f"r{rep}_sc_{j}_{kc}_{h}")
                    for n in range(2):
                        nc.tensor.matmul(
                            scps[:, n * 512:(n + 1) * 512],
                            kt[o:o + 64, kc * P:(kc + 1) * P],
                            qt[o:o + 64, n * 512:(n + 1) * 512],
                            start=True, stop=True, tile_position=(o, 0))
                    expt = exp_pool.tile([P, 1024], BF16, tag="expt",
                                         name=f"r{rep}_ex_{j}_{kc}_{h}")
                    zt = small_pool.tile([P, 1], F32, tag="z",
                                         name=f"r{rep}_z_{j}_{kc}_{h}")
                    nc.scalar.activation(expt[:], scps[:], AF.Exp,
                                         scale=0.1, accum_out=zt[:])
                    nc.vector.reciprocal(zt[:], zt[:])
                    vsc = small_pool.tile([P, 64], BF16, tag="vsc",
                                          name=f"r{rep}_vs_{j}_{kc}_{h}")
                    head = 2 * j + h
                    nc.vector.tensor_scalar_mul(
                        vsc[:], v_sb[kc][:, head * 64:(head + 1) * 64], zt[:])
                    return expt, vsc

                def emit_av(avp, kc, h, expt, vsc):
                    o = 64 * h
                    for n in range(2):
                        nc.tensor.matmul(
                            avp[o:o + 64, n * 512:(n + 1) * 512],
                            vsc[:], expt[:, n * 512:(n + 1) * 512],
                            start=(kc == 0), stop=(kc == NS - 1),
                            tile_position=(0, o))

                pending_stats = []   # (m, jpair) bn_stats to spread over kc
                for j in range(NE):
                    qt, kt = qk_sb.pop(j)
                    avp = ps_av.tile([P, 1024], F32, tag="av",
                                     name=f"r{rep}_av_{j}")
                    pend = None   # (kc, [(h, expt, vsc)...]) awaiting av
                    for kc in range(NS):
                        if j == 0:
                            while len(v_sb) < min(kc + 4, NS):
                                v_sb.append(emit_v(len(v_sb)))
                            if kc == 5:
                                wv_scope.close()
                            if kc == 1:
                                qk_sb[1] = emit_qk(1)
                        cur = []
                        for h in range(2):
                            cur.append((h,) + emit_sc_exp(j, kc, h, qt, kt))
                        if pend is not None:
                            pkc, items = pend
                            for h, expt, vsc in items:
                                emit_av(avp, pkc, h, expt, vsc)
                        pend = (kc, cur)
                        # interleave next-j qk matmuls into the kc loop
                        if kc == 3 and j + 2 < NE:
                            qk_sb[j + 2] = emit_qk(j + 2)
                        if pending_stats:
                            m, jp = pending_stats.pop()
                            nc.vector.bn_stats(
                                st1[m][:, jp, :],
                                att_all[:, m, (2 * jp) * P:(2 * jp + 2) * P])
                    pkc, items = pend
                    for h, expt, vsc in items:
                        emit_av(avp, pkc, h, expt, vsc)

                    attjT = attjT_pool.tile([P, S], BF16, tag="attjT",
                                            name=f"r{rep}_attjT_{j}")
                    nc.vector.tensor_copy(attjT[:], avp[:])
                    nc.sync.dma_start_transpose(
                        att_all[:, :, j * P:(j + 1) * P], attjT[:])
                    if j % 2 == 1:
                        if j < NE - 1:
                            pending_stats = [(m, j // 2) for m in range(NS)]
                        else:
                            for m in range(NS):
                                nc.vector.bn_stats(
                                    st1[m][:, j // 2, :],
                                    att_all[:, m, (j - 1) * P:(j + 1) * P])
                    # residual prep (bf16): a1bf[m] = x[m] + be1
                    nc.vector.tensor_tensor(a1bf[j][:], a1bf[j][:], be1_b[:],
                                            ALU.add)
                    if j == NE - 2:
                        scopeA.close()   # free xT/wq/wk for w1 prefetch

                # -------------------------------------- LN1 + adn1T ----
                mv1, rstd1 = [], []
                for m in range(NS):
                    mv = st_pool.tile([P, 2], F32, tag="mv1",
                                      name=f"r{rep}_mv1_{m}")
                    nc.vector.bn_aggr(mv[:], st1[m][:])
                    rstd = st_pool.tile([P, 1], F32, tag="rstd1",
                                        name=f"r{rep}_rstd1_{m}")
                    nc.scalar.activation(rstd[:], mv[:, 1:2], AF.Sqrt,
                                         bias=eps_t[:])
                    nc.vector.reciprocal(rstd[:], rstd[:])
                    mv1.append(mv)
                    rstd1.append(rstd)
                for m in range(NS):
                    u = u_pool.tile([P, E], BF16, tag="u", name=f"r{rep}_u_{m}")
                    nc.vector.tensor_scalar(u[:], att_all[:, m, :],
                                            mv1[m][:, 0:1], rstd1[m][:],
                                            ALU.subtract, ALU.mult)
                    nc.vector.tensor_tensor(u[:], u[:], g1_b[:], ALU.mult)
                    nc.vector.tensor_tensor(a1bf[m][:], a1bf[m][:], u[:],
                                            ALU.add)
                    nc.sync.dma_start_transpose(
                        a1T[m // 4][:, :, (m % 4) * P:(m % 4 + 1) * P],
                        a1bf[m][:])

            # ------------------------------------------------ FFN ----
            with ExitStack() as ffn:
                hT_pool = ffn.enter_context(tc.tile_pool(name="hTp",
                                                         bufs=2 * NF))
                ps_f = ffn.enter_context(
                    tc.tile_pool(name="ps_f", bufs=4, space="PSUM"))
                w2a_pool = ffn.enter_context(tc.tile_pool(name="w2ap",
                                                          bufs=NF // 2))
                scopeW1 = ExitStack()     # w1 halves; closed after FFN1
                ffn.enter_context(scopeW1)
                w1a_pool = scopeW1.enter_context(
                    tc.tile_pool(name="w1ap", bufs=NE))
                w1b_pool = scopeW1.enter_context(
                    tc.tile_pool(name="w1bp", bufs=NE))
                w1a_sb, w1b_sb, w2_sb = [], [], []
                for c in range(NE):
                    t = w1a_pool.tile([P, F // 2], BF16, tag="w1a",
                                      name=f"r{rep}_w1a_{c}")
                    nc.gpsimd.dma_start(t[:], w1_c[c][:, 0:F // 2])
                    w1a_sb.append(t)
                for c in range(NE):
                    t = w1b_pool.tile([P, F // 2], BF16, tag="w1b",
                                      name=f"r{rep}_w1b_{c}")
                    nc.gpsimd.dma_start(t[:], w1_c[c][:, F // 2:F])
                    w1b_sb.append(t)
                for c in range(NF // 2):
                    t = w2a_pool.tile([P, E], BF16, tag="w2a",
                                      name=f"r{rep}_w2_{c}")
                    nc.gpsimd.dma_start(t[:], w2_c[c])
                    w2_sb.append(t)

                # hT split by s-half (FFN2 m<4 only needs the n=0 half)
                hT = [[hT_pool.tile([P, S // 2], BF16, tag="hT",
                                    name=f"r{rep}_hT_{f}_{n}") for n in range(2)]
                      for f in range(NF)]

                for n in range(2):
                    for f in range(NF):
                        ps = ps_f.tile([P, 512], F32, tag="f",
                                       name=f"r{rep}_psf_{f}_{n}")
                        w_half = w1a_sb if f < NF // 2 else w1b_sb
                        fo = f if f < NF // 2 else f - NF // 2
                        for c in range(NE):
                            nc.tensor.matmul(
                                ps[:],
                                w_half[c][:, fo * P:(fo + 1) * P],
                                a1T[n][:, c, :],
                                start=(c == 0), stop=(c == NE - 1))
                        nc.scalar.activation(hT[f][n][:], ps[:], AF.Gelu,
                                             bias=b1_sb[:, f:f + 1])

                scopeW1.close()   # free w1 halves; a1T no longer needed
                with ExitStack() as ffn2:
                    ps_2 = ffn2.enter_context(
                        tc.tile_pool(name="ps_2", bufs=2, space="PSUM"))
                    w2b_pool = ffn2.enter_context(tc.tile_pool(name="w2bp",
                                                               bufs=NF // 2))
                    bc2_pool = ffn2.enter_context(tc.tile_pool(name="bc2p",
                                                               bufs=1))
                    ff_pool = ffn2.enter_context(tc.tile_pool(name="ffp",
                                                              bufs=2))
                    u2_pool = ffn2.enter_context(tc.tile_pool(name="u2p",
                                                              bufs=2))
                    st2_pool = ffn2.enter_context(tc.tile_pool(name="st2p",
                                                               bufs=6))
                    out_pool = ffn2.enter_context(tc.tile_pool(name="outp",
                                                               bufs=2))
                    for c in range(NF // 2, NF):
                        t = w2b_pool.tile([P, E], BF16, tag="w2b",
                                          name=f"r{rep}_w2_{c}")
                        nc.gpsimd.dma_start(t[:], w2_c[c])
                        w2_sb.append(t)

                    b2_b = bc2_pool.tile([P, E], F32, tag="b2b")
                    gff_b = bc2_pool.tile([P, E], F32, tag="gffb")
                    bff_b = bc2_pool.tile([P, E], F32, tag="bffb")
                    g2_b = bc2_pool.tile([P, E], F32, tag="g2b")
                    be2_b = bc2_pool.tile([P, E], F32, tag="be2b")
                    nc.gpsimd.dma_start(b2_b[:], _bcast_ap(b2, E))
                    nc.gpsimd.dma_start(gff_b[:], _bcast_ap(gff, E))
                    nc.gpsimd.dma_start(bff_b[:], _bcast_ap(bff, E))
                    nc.gpsimd.dma_start(g2_b[:], _bcast_ap(g2, E))
                    nc.gpsimd.dma_start(be2_b[:], _bcast_ap(be2, E))

                    def ln_tile(src_ap, dst_ap, g_b, b_b, nm):
                        stats = st2_pool.tile([P, 2, 6], F32, tag="st2",
                                              name=f"r{rep}_st2_{nm}")
                        mv = st2_pool.tile([P, 2], F32, tag="mv2",
                                           name=f"r{rep}_mv2_{nm}")
                        nc.vector.bn_stats(stats[:, 0, :], src_ap[:, 0:512])
                        nc.vector.bn_stats(stats[:, 1, :], src_ap[:, 512:1024])
                        nc.vector.bn_aggr(mv[:], stats[:])
                        rstd = st2_pool.tile([P, 1], F32, tag="rstd2",
                                             name=f"r{rep}_rstd2_{nm}")
                        nc.scalar.activation(rstd[:], mv[:, 1:2], AF.Sqrt,
                                             bias=eps_t[:])
                        nc.vector.reciprocal(rstd[:], rstd[:])
                        nc.vector.tensor_scalar(dst_ap, src_ap, mv[:, 0:1],
                                                rstd[:], ALU.subtract,
                                                ALU.mult)
                        nc.vector.tensor_tensor(dst_ap, dst_ap, g_b[:],
                                                ALU.mult)
                        nc.vector.tensor_tensor(dst_ap, dst_ap, b_b[:],
                                                ALU.add)

                    for m in range(NS):
                        ps = ps_2.tile([P, 1024], F32, tag="o",
                                       name=f"r{rep}_ps2_{m}")
                        half, mm = (0, m) if m < 4 else (1, m - 4)
                        for c in range(NF):
                            for n in range(2):
                                nc.tensor.matmul(
                                    ps[:, n * 512:(n + 1) * 512],
                                    hT[c][half][:, mm * P:(mm + 1) * P],
                                    w2_sb[c][:, n * 512:(n + 1) * 512],
                                    start=(c == 0), stop=(c == NF - 1))
                        ffb = ff_pool.tile([P, E], F32, tag="ffb",
                                           name=f"r{rep}_ffb_{m}")
                        nc.vector.tensor_tensor(ffb[:], ps[:], b2_b[:],
                                                ALU.add)
                        ln_tile(ffb[:], ffb[:], gff_b, bff_b, f"ff_{m}")
                        u2 = u2_pool.tile([P, E], F32, tag="u2",
                                          name=f"r{rep}_u2_{m}")
                        ln_tile(ffb[:], u2[:], g2_b, be2_b, f"l2_{m}")
                        ot = out_pool.tile([P, E], F32, tag="ot",
                                           name=f"r{rep}_ot_{m}")
                        nc.vector.tensor_tensor(ot[:], u2[:], a1bf[m][:],
                                                ALU.add)
                        nc.sync.dma_start(out_c[m], ot[:])

    _split_sync_waits(nc)
    nc.finalize()
    return nc


_NC = {}


def _get_nc(reps=1, mode="full"):
    key = (reps, mode)
    if key not in _NC:
        _NC[key] = build(reps, mode)
    return _NC[key]


def make_in_maps(inputs):
    bf = ml_dtypes.bfloat16
    x = np.ascontiguousarray(np.asarray(inputs["x"], dtype=np.float32))
    shared = {
        "wq": np.ascontiguousarray(np.asarray(inputs["Wq"], np.float32).astype(bf)),
        "wk": np.ascontiguousarray(np.asarray(inputs["Wk"], np.float32).astype(bf)),
        "wv": np.ascontiguousarray(np.asarray(inputs["Wv"], np.float32).astype(bf)),
        "w1": np.ascontiguousarray(np.asarray(inputs["W1"], np.float32).astype(bf)),
        "w2": np.ascontiguousarray(np.asarray(inputs["W2"], np.float32).astype(bf)),
        "bq": np.asarray(inputs["bq"], np.float32),
        "bk": np.asarray(inputs["bk"], np.float32),
        "bv": np.asarray(inputs["bv"], np.float32),
        "b1": np.asarray(inputs["b1"], np.float32),
        "b2": np.asarray(inputs["b2"], np.float32),
        "g1": np.asarray(inputs["ln1_g"], np.float32),
        "be1": np.asarray(inputs["ln1_b"], np.float32),
        "gff": np.asarray(inputs["ln_ff_g"], np.float32),
        "bff": np.asarray(inputs["ln_ff_b"], np.float32),
        "g2": np.asarray(inputs["ln2_g"], np.float32),
        "be2": np.asarray(inputs["ln2_b"], np.float32),
    }
    in_maps = []
    for i in range(B):
        m = dict(shared)
        m["x"] = np.ascontiguousarray(x[i])
        m["xT"] = np.ascontiguousarray(x[i].T.astype(bf))
        in_maps.append(m)
    return in_maps


def kernel(**inputs):
    nc = _get_nc()
    in_maps = make_in_maps(inputs)
    res = run_bass_kernel_spmd(nc, in_maps, list(range(B)))
    return np.stack([res.results[i]["out"] for i in range(B)], axis=0)


# revision 30
# speedup vs baseline: 1.6320x; 1.2467x over previous
"""Trainium2 Bass kernel for nn_Block_52527450030210 (dense transformer block).

B=8, S=1024, E=1024, H=16 heads (D=64), F=4096. Data-parallel: batch element i
runs on core i (no collectives). Matmuls in bf16 with fp32 PSUM accumulation;
LayerNorms/softmax in fp32. Softmax is over the QUERY axis (dim=-2), so scores
are computed transposed ([k, q] layout) making the softmax reduction a
free-axis reduction, and 1/Z folds into v (Z is per contraction-index k).

v2 schedule: q/k projections are software-pipelined into the attention
k-tile loop to keep the PE array dense (HAM stays warm), softmax exps are
[128,1024]-wide (half the ACT instruction overhead), all transposes go
through the DMA xbar instead of the PE+ACT path, LN1 statistics are
accumulated incrementally during attention, FFN weights prefetch on the
SWDGE queue in halves sized to fit SBUF, and FFN1/FFN2 form one continuous
PE stream. adn1 stays in SBUF (bf16) instead of round-tripping through DRAM.

Self-contained: hardcodes shapes, includes the walrus single-sync-wait
workaround (this container's walrus accepts only one sync-wait per
instruction; Tile emits several, so extra waits are hoisted onto same-engine
NoOps).
"""

import numpy as np
import ml_dtypes

import concourse.bass as bass
import concourse.mybir as mybir
import concourse.tile as tile
from concourse.bass_utils import run_bass_kernel_spmd
from concourse.vector_clock import ScopedClock
from contextlib import ExitStack

F32 = mybir.dt.float32
BF16 = mybir.dt.bfloat16
AF = mybir.ActivationFunctionType
ALU = mybir.AluOpType

B, S, E, H, D, F = 8, 1024, 1024, 16, 64, 4096
P = 128
NE = E // P   # 8 e-chunks
NS = S // P   # 8 s-tiles
NF = F // P   # 32 f-tiles
EPS = 1e-5

# ---------------------------------------------------------------- waitfix ---

_wf_counter = [0]


def _wait_nop(nc, engine, wait, debug):
    _wf_counter[0] += 1
    nop = mybir.InstNoOp(
        name=f"I-wsplit-{_wf_counter[0]}", ins=[], outs=[], debug=debug,
        bass_nofuse=True,
    )
    nop.engine = engine
    nop.sync_info = mybir.SyncInfo(on_wait=[wait], on_update=[])
    nc.register_instruction(nop, overwrite=True)
    return nop


def _split_sync_waits(nc):
    for _name, bb in nc.bb_map.items():
        if not hasattr(bb, "instructions"):
            bb = bb.bb
        il = bb.instructions
        changed = False
        new = []
        for inst in il:
            si = inst.sync_info
            if si is not None and si.on_wait and len(si.on_wait) > 1:
                waits = list(si.on_wait)
                for w in waits[:-1]:
                    new.append(_wait_nop(nc, inst.engine, w, inst.debug))
                si.on_wait = waits[-1:]
                changed = True
            new.append(inst)
        if changed:
            bb.instructions = new


def _patched_drain_and_barrier(self, tick_clock, wait_clock):
    nop0 = self.nc.sync.nop(nofuse=True, hint="tail_wait")
    wait_clock.add_sem_waits(nop0.ins, ScopedClock({None: tick_clock.global_clock}))
    si = nop0.ins.sync_info
    waits = list(si.on_wait) if si and si.on_wait else []
    if len(waits) > 1:
        si.on_wait = waits[:1]
        rest = waits[1:]
        while rest:
            nop = self.nc.sync.nop(nofuse=True, hint="tail_wait")
            nop.ins.sync_info = mybir.SyncInfo(on_wait=rest[:1], on_update=[])
            rest = rest[1:]
    self.nc.sync.drain()
    self.nc.all_engine_barrier()
    assert self.sems is not None
    popped = self.nc._tile_sem_poison_stack.pop()
    assert popped is self._sem_poison
    self.nc.clear_and_free_semaphores(list(self.sems.allocated().values()))
    self.nc.all_engine_barrier()


tile.TileContext._drain_and_barrier = _patched_drain_and_barrier

# ----------------------------------------------------------------- build -----


def _bcast_ap(dram_t, n):
    """AP that DMA-broadcasts a [n] DRAM vector to [128, n] (partition step 0)."""
    return bass.AP(tensor=dram_t, offset=0, ap=[[0, P], [1, n]])


def build(reps=1, mode="full"):
    nc = bass.Bass()

    xT = nc.dram_tensor("xT", [E, S], BF16, kind="ExternalInput")
    x_f = nc.dram_tensor("x", [S, E], F32, kind="ExternalInput")
    wq = nc.dram_tensor("wq", [E, E], BF16, kind="ExternalInput")
    wk = nc.dram_tensor("wk", [E, E], BF16, kind="ExternalInput")
    wv = nc.dram_tensor("wv", [E, E], BF16, kind="ExternalInput")
    w1 = nc.dram_tensor("w1", [E, F], BF16, kind="ExternalInput")
    w2 = nc.dram_tensor("w2", [F, E], BF16, kind="ExternalInput")
    bqt = nc.dram_tensor("bqt", [P, NE], F32, kind="ExternalInput")
    bkt = nc.dram_tensor("bkt", [P, NE], F32, kind="ExternalInput")
    bv = nc.dram_tensor("bv", [E], F32, kind="ExternalInput")
    b1t = nc.dram_tensor("b1t", [P, NF], F32, kind="ExternalInput")
    b2 = nc.dram_tensor("b2", [E], F32, kind="ExternalInput")
    g1 = nc.dram_tensor("g1", [E], F32, kind="ExternalInput")
    be1 = nc.dram_tensor("be1", [E], F32, kind="ExternalInput")
    gff = nc.dram_tensor("gff", [E], F32, kind="ExternalInput")
    bff = nc.dram_tensor("bff", [E], F32, kind="ExternalInput")
    g2 = nc.dram_tensor("g2", [E], F32, kind="ExternalInput")
    be2 = nc.dram_tensor("be2", [E], F32, kind="ExternalInput")

    xT_c = xT.rearrange("(c p) s -> c p s", p=P)
    x_c = x_f.rearrange("(m p) e -> m p e", p=P)
    wq_c = wq.rearrange("(c p) e -> c p e", p=P)
    wk_c = wk.rearrange("(c p) e -> c p e", p=P)
    wv_c = wv.rearrange("(c p) e -> c p e", p=P)
    w1_c = w1.rearrange("(c p) f -> c p f", p=P)
    w2_c = w2.rearrange("(c p) e -> c p e", p=P)

    with tile.TileContext(nc) as tc:
      for rep in range(reps):
        out_d = nc.dram_tensor("out" if rep == 0 else f"out_r{rep}",
                               [S, E], F32, kind="ExternalOutput")
        out_c = out_d.rearrange("(m p) e -> m p e", p=P)
        with ExitStack() as top:
            const = top.enter_context(tc.tile_pool(name="const", bufs=1))
            eps_t = const.tile([P, 1], F32)
            nc.vector.memset(eps_t[:], EPS)
            bq_sb = const.tile([P, NE], F32)
            bk_sb = const.tile([P, NE], F32)
            b1_sb = const.tile([P, NF], F32)
            nc.sync.dma_start(bq_sb[:], bqt[:, :])
            nc.sync.dma_start(bk_sb[:], bkt[:, :])
            nc.sync.dma_start(b1_sb[:], b1t[:, :])

            # residual (bf16) and its transpose live across the whole rep
            a1_pool = top.enter_context(tc.tile_pool(name="a1p", bufs=NS))
            a1T_pool = top.enter_context(tc.tile_pool(name="a1Tp", bufs=2))
            a1bf = [a1_pool.tile([P, E], BF16, tag="a1", name=f"r{rep}_a1_{m}")
                    for m in range(NS)]
            # adn1T split by s-half so FFN1 n=0 only depends on LN1 of m=0..3;
            # [P, c, s] group-fold layout filled by one xbar transpose per m
            a1T = [a1T_pool.tile([P, NE, S // 2], BF16, tag="a1T",
                                 name=f"r{rep}_a1T_{n}") for n in range(2)]

            # ---------------------------------------------- attention ----
            with ExitStack() as attn:
                att_pool = attn.enter_context(tc.tile_pool(name="attp",
                                                           bufs=1))
                st_pool = attn.enter_context(tc.tile_pool(name="stp", bufs=NS))
                bc1_pool = attn.enter_context(tc.tile_pool(name="bc1p",
                                                           bufs=1))
                v_pool = attn.enter_context(tc.tile_pool(name="vp", bufs=NS))
                qk_pool = attn.enter_context(tc.tile_pool(name="qkp", bufs=6))
                exp_pool = attn.enter_context(tc.tile_pool(name="expp",
                                                           bufs=6))
                small_pool = attn.enter_context(tc.tile_pool(name="smallp",
                                                             bufs=8))
                wqkj_pool = attn.enter_context(tc.tile_pool(name="wqkjp",
                                                            bufs=6))
                attjT_pool = attn.enter_context(tc.tile_pool(name="attjTp",
                                                             bufs=1))
                u_pool = attn.enter_context(tc.tile_pool(name="up", bufs=1))

                # att in [s, e] layout: one tile, group-fold m = dim 1
                att_all = att_pool.tile([P, NS, E], BF16,
                                        name=f"r{rep}_att_all")
                st1 = [st_pool.tile([P, 4, 6], F32, tag="st1",
                                    name=f"r{rep}_st1_{m}")
                       for m in range(NS)]

                bv_b = bc1_pool.tile([P, E], BF16, tag="bvb")
                g1_b = bc1_pool.tile([P, E], BF16, tag="g1b")
                be1_b = bc1_pool.tile([P, E], BF16, tag="be1b")
                nc.gpsimd.dma_start(bv_b[:], _bcast_ap(bv, E))
                nc.gpsimd.dma_start(g1_b[:], _bcast_ap(g1, E))
                nc.gpsimd.dma_start(be1_b[:], _bcast_ap(be1, E))

                scopeA = ExitStack()      # xT (freed after last qk)
                attn.enter_context(scopeA)
                xT_pool = scopeA.enter_context(tc.tile_pool(name="xTp",
                                                            bufs=NE))

                # per-j [P, c, 128] slices of wq/wk; full tensors stay in DRAM
                wq_j = wq.rearrange("(c p) e -> p c e", p=P)
                wk_j = wk.rearrange("(c p) e -> p c e", p=P)

                # xT + wv first (v starts ASAP)
                xT_sb = []
                wv_scope = ExitStack()
                attn.enter_context(wv_scope)
                wv_pool = wv_scope.enter_context(tc.tile_pool(name="wvp",
                                                              bufs=NE))
                wv_sb = []
                for c in range(NE):
                    t = xT_pool.tile([P, S], BF16, tag="xT",
                                     name=f"r{rep}_xT_{c}")
                    nc.sync.dma_start(t[:], xT_c[c])
                    xT_sb.append(t)
                    tv = wv_pool.tile([P, E], BF16, tag="wv",
                                      name=f"r{rep}_wv_{c}")
                    nc.sync.dma_start(tv[:], wv_c[c])
                    wv_sb.append(tv)
                for m in range(NS):
                    nc.gpsimd.dma_start(a1bf[m][:], x_c[m])

                ps_big = attn.enter_context(
                    tc.tile_pool(name="ps_big", bufs=3, space="PSUM"))
                ps_av = attn.enter_context(
                    tc.tile_pool(name="ps_av", bufs=1, space="PSUM"))

                def emit_qk(j):
                    """qT[j], kT[j] (bf16 [P,S]) via shared ps_big pool."""
                    qt = qk_pool.tile([P, S], BF16, tag="qkT",
                                      name=f"r{rep}_qT_{j}")
                    kt = qk_pool.tile([P, S], BF16, tag="qkT",
                                      name=f"r{rep}_kT_{j}")
                    for di, (dst, w_dram, bias) in enumerate(
                            ((qt, wq_j, bq_sb), (kt, wk_j, bk_sb))):
                        wj = wqkj_pool.tile([P, NE, P], BF16, tag="wqkj",
                                            name=f"r{rep}_wj_{j}_{di}")
                        nc.sync.dma_start(wj[:],
                                          w_dram[:, :, j * P:(j + 1) * P])
                        ps = ps_big.tile([P, 1024], F32, tag="big",
                                         name=f"r{rep}_psqk_{j}_{di}")
                        for n in range(2):
                            for c in range(NE):
                                nc.tensor.matmul(
                                    ps[:, n * 512:(n + 1) * 512],
                                    wj[:, c, :],
                                    xT_sb[c][:, n * 512:(n + 1) * 512],
                                    start=(c == 0), stop=(c == NE - 1))
                        nc.vector.tensor_scalar_add(dst[:], ps[:],
                                                    bias[:, j:j + 1])
                    return qt, kt

                def emit_v(m):
                    vt = v_pool.tile([P, E], BF16, tag="v", name=f"r{rep}_v_{m}")
                    ps = ps_big.tile([P, 1024], F32, tag="big",
                                     name=f"r{rep}_psv_{m}")
                    for n in range(2):
                        for c in range(NE):
                            nc.tensor.matmul(
                                ps[:, n * 512:(n + 1) * 512],
                                xT_sb[c][:, m * P:(m + 1) * P],
                                wv_sb[c][:, n * 512:(n + 1) * 512],
                                start=(c == 0), stop=(c == NE - 1))
                    nc.vector.tensor_tensor(vt[:], ps[:], bv_b[:], ALU.add)
                    return vt

                v_sb = [emit_v(m) for m in range(3)]
                qk_sb = {0: emit_qk(0)}

                # qk for later j: spread as 4-MM quarter-groups, one per kc
                # step, so the PE never crowds out the ACT-paced softmax.
                qk_pending = []   # FIFO of (jt, di, qd)
                qk_state = {}

                def start_qk(jt):
                    st = {
                        "qt": qk_pool.tile([P, S], BF16, tag="qkT",
                                           name=f"r{rep}_qT_{jt}"),
                        "kt": qk_pool.tile([P, S], BF16, tag="qkT",
                                           name=f"r{rep}_kT_{jt}"),
                    }
                    qk_state[jt] = st
                    qk_pending.extend((jt, di, qd)
                                      for di in range(2) for qd in range(4))

                def emit_qk_quarter(jt, di, qd):
                    st = qk_state[jt]
                    w_dram, bias = ((wq_j, bq_sb), (wk_j, bk_sb))[di]
                    if qd == 0:
                        wj = wqkj_pool.tile([P, NE, P], BF16, tag="wqkj",
                                            name=f"r{rep}_wj_{jt}_{di}")
                        nc.sync.dma_start(wj[:],
                                          w_dram[:, :, jt * P:(jt + 1) * P])
                        st["wj"] = wj
                        st["ps"] = ps_big.tile([P, 1024], F32, tag="big",
                                               name=f"r{rep}_psqk_{jt}_{di}")
                    n, ch = qd // 2, qd % 2
                    ps = st["ps"]
                    for c in range(4 * ch, 4 * ch + 4):
                        nc.tensor.matmul(
                            ps[:, n * 512:(n + 1) * 512],
                            st["wj"][:, c, :],
                            xT_sb[c][:, n * 512:(n + 1) * 512],
                            start=(c == 0), stop=(c == NE - 1))
                    if qd == 3:
                        dst = st["qt"] if di == 0 else st["kt"]
                        nc.vector.tensor_scalar_add(dst[:], ps[:],
                                                    bias[:, jt:jt + 1])
                        del st["ps"], st["wj"]

                start_qk(1)

                def emit_sc_exp(j, kc, h, qt, kt):
                    o = 64 * h
                    scps = ps_big.tile([P, 1024], F32, tag="big",
                                       name=f"r{rep}_sc_{j}_{kc}_{h}")
                    for n in range(2):
                        nc.tensor.matmul(
                            scps[:, n * 512:(n + 1) * 512],
                            kt[o:o + 64, kc * P:(kc + 1) * P],
                            qt[o:o + 64, n * 512:(n + 1) * 512],
                            start=True, stop=True, tile_position=(o, 0))
                    expt = exp_pool.tile([P, 1024], BF16, tag="expt",
                                         name=f"r{rep}_ex_{j}_{kc}_{h}")
                    zt = small_pool.tile([P, 1], F32, tag="z",
                                         name=f"r{rep}_z_{j}_{kc}_{h}")
                    nc.scalar.activation(expt[:], scps[:], AF.Exp,
                                         scale=0.1, accum_out=zt[:])
                    nc.vector.reciprocal(zt[:], zt[:])
                    vsc = small_pool.tile([P, 64], BF16, tag="vsc",
                                          name=f"r{rep}_vs_{j}_{kc}_{h}")
                    head = 2 * j + h
                    nc.vector.tensor_scalar_mul(
                        vsc[:], v_sb[kc][:, head * 64:(head + 1) * 64], zt[:])
                    return expt, vsc

                def emit_av(avp, kc, h, expt, vsc):
                    o = 64 * h
                    for n in range(2):
                        nc.tensor.matmul(
                            avp[o:o + 64, n * 512:(n + 1) * 512],
                            vsc[:], expt[:, n * 512:(n + 1) * 512],
                            start=(kc == 0), stop=(kc == NS - 1),
                            tile_position=(0, o))

                pending_stats = []   # (m, jpair) bn_stats to spread over kc

                def epilogue(j, avp):
                    """Drain avp(j) to att_all cols j (runs inside j+1 kc0)."""
                    attjT = attjT_pool.tile([P, S], BF16, tag="attjT",
                                            name=f"r{rep}_attjT_{j}")
                    nc.vector.tensor_copy(attjT[:], avp[:])
                    nc.sync.dma_start_transpose(
                        att_all[:, :, j * P:(j + 1) * P], attjT[:])
                    if j % 2 == 1:
                        if j < NE - 1:
                            pending_stats.extend(
                                (m, j // 2) for m in range(NS))
                        else:
                            for m in range(NS):
                                nc.vector.bn_stats(
                                    st1[m][:, j // 2, :],
                                    att_all[:, m, (j - 1) * P:(j + 1) * P])

                pend = None   # (avp, kc, [(h, expt, vsc)...], j)
                for j in range(NE):
                    qt, kt = (qk_sb.pop(0) if j == 0
                              else (qk_state[j]["qt"], qk_state[j]["kt"]))
                    if j + 2 < NE:
                        start_qk(j + 2)
                    avp = ps_av.tile([P, 1024], F32, tag="av",
                                     name=f"r{rep}_av_{j}")
                    for kc in range(NS):
                        if j == 0:
                            while len(v_sb) < min(kc + 4, NS):
                                v_sb.append(emit_v(len(v_sb)))
                            if kc == 5:
                                wv_scope.close()
                        cur = []
                        for h in range(2):
                            cur.append((h,) + emit_sc_exp(j, kc, h, qt, kt))
                        if pend is not None:
                            pavp, pkc, items, pj = pend
                            for h, expt, vsc in items:
                                emit_av(pavp, pkc, h, expt, vsc)
                            if pkc == NS - 1:
                                epilogue(pj, pavp)
                        pend = (avp, kc, cur, j)
                        if qk_pending:
                            emit_qk_quarter(*qk_pending.pop(0))
                        if pending_stats:
                            m, jp = pending_stats.pop()
                            nc.vector.bn_stats(
                                st1[m][:, jp, :],
                                att_all[:, m, (2 * jp) * P:(2 * jp + 2) * P])
                    # residual prep (bf16): a1bf[j] = x[j] + be1
                    nc.vector.tensor_tensor(a1bf[j][:], a1bf[j][:], be1_b[:],
                                            ALU.add)
                    if j == NE - 2:
                        scopeA.close()   # free xT for w1 prefetch
                pavp, pkc, items, pj = pend
                for h, expt, vsc in items:
                    emit_av(pavp, pkc, h, expt, vsc)
                epilogue(pj, pavp)

                # -------------------------------------- LN1 + adn1T ----
                mv1, rstd1 = [], []
                for m in range(NS):
                    mv = st_pool.tile([P, 2], F32, tag="mv1",
                                      name=f"r{rep}_mv1_{m}")
                    nc.vector.bn_aggr(mv[:], st1[m][:])
                    rstd = st_pool.tile([P, 1], F32, tag="rstd1",
                                        name=f"r{rep}_rstd1_{m}")
                    nc.scalar.activation(rstd[:], mv[:, 1:2], AF.Sqrt,
                                         bias=eps_t[:])
                    nc.vector.reciprocal(rstd[:], rstd[:])
                    mv1.append(mv)
                    rstd1.append(rstd)
                for m in range(NS):
                    u = u_pool.tile([P, E], BF16, tag="u", name=f"r{rep}_u_{m}")
                    nc.vector.tensor_scalar(u[:], att_all[:, m, :],
                                            mv1[m][:, 0:1], rstd1[m][:],
                                            ALU.subtract, ALU.mult)
                    nc.vector.tensor_tensor(u[:], u[:], g1_b[:], ALU.mult)
                    nc.vector.tensor_tensor(a1bf[m][:], a1bf[m][:], u[:],
                                            ALU.add)
                    nc.sync.dma_start_transpose(
                        a1T[m // 4][:, :, (m % 4) * P:(m % 4 + 1) * P],
                        a1bf[m][:])

            # ------------------------------------------------ FFN ----
            with ExitStack() as ffn:
                hT_pool = ffn.enter_context(tc.tile_pool(name="hTp",
                                                         bufs=2 * NF))
                ps_f = ffn.enter_context(
                    tc.tile_pool(name="ps_f", bufs=4, space="PSUM"))
                w2a_pool = ffn.enter_context(tc.tile_pool(name="w2ap",
                                                          bufs=NF // 2))
                scopeW1 = ExitStack()     # w1 halves; closed after FFN1
                ffn.enter_context(scopeW1)
                w1a_pool = scopeW1.enter_context(
                    tc.tile_pool(name="w1ap", bufs=NE))
                w1b_pool = scopeW1.enter_context(
                    tc.tile_pool(name="w1bp", bufs=NE))
                w1a_sb, w1b_sb, w2_sb = [], [], []
                for c in range(NE):
                    t = w1a_pool.tile([P, F // 2], BF16, tag="w1a",
                                      name=f"r{rep}_w1a_{c}")
                    nc.gpsimd.dma_start(t[:], w1_c[c][:, 0:F // 2])
                    w1a_sb.append(t)
                for c in range(NE):
                    t = w1b_pool.tile([P, F // 2], BF16, tag="w1b",
                                      name=f"r{rep}_w1b_{c}")
                    nc.gpsimd.dma_start(t[:], w1_c[c][:, F // 2:F])
                    w1b_sb.append(t)
                for c in range(NF // 2):
                    t = w2a_pool.tile([P, E], BF16, tag="w2a",
                                      name=f"r{rep}_w2_{c}")
                    nc.gpsimd.dma_start(t[:], w2_c[c])
                    w2_sb.append(t)

                # hT split by s-half (FFN2 m<4 only needs the n=0 half)
                hT = [[hT_pool.tile([P, S // 2], BF16, tag="hT",
                                    name=f"r{rep}_hT_{f}_{n}") for n in range(2)]
                      for f in range(NF)]

                for n in range(2):
                    for f in range(NF):
                        ps = ps_f.tile([P, 512], F32, tag="f",
                                       name=f"r{rep}_psf_{f}_{n}")
                        w_half = w1a_sb if f < NF // 2 else w1b_sb
                        fo = f if f < NF // 2 else f - NF // 2
                        for c in range(NE):
                            nc.tensor.matmul(
                                ps[:],
                                w_half[c][:, fo * P:(fo + 1) * P],
                                a1T[n][:, c, :],
                                start=(c == 0), stop=(c == NE - 1))
                        nc.scalar.activation(hT[f][n][:], ps[:], AF.Gelu,
                                             bias=b1_sb[:, f:f + 1])

                scopeW1.close()   # free w1 halves; a1T no longer needed
                with ExitStack() as ffn2:
                    ps_2 = ffn2.enter_context(
                        tc.tile_pool(name="ps_2", bufs=2, space="PSUM"))
                    w2b_pool = ffn2.enter_context(tc.tile_pool(name="w2bp",
                                                               bufs=NF // 2))
                    bc2_pool = ffn2.enter_context(tc.tile_pool(name="bc2p",
                                                               bufs=1))
                    ff_pool = ffn2.enter_context(tc.tile_pool(name="ffp",
                                                              bufs=2))
                    u2_pool = ffn2.enter_context(tc.tile_pool(name="u2p",
                                                              bufs=2))
                    st2_pool = ffn2.enter_context(tc.tile_pool(name="st2p",
                                                               bufs=6))
                    out_pool = ffn2.enter_context(tc.tile_pool(name="outp",
                                                               bufs=2))
                    for c in range(NF // 2, NF):
                        t = w2b_pool.tile([P, E], BF16, tag="w2b",
                                          name=f"r{rep}_w2_{c}")
                        nc.gpsimd.dma_start(t[:], w2_c[c])
                        w2_sb.append(t)

                    b2_b = bc2_pool.tile([P, E], F32, tag="b2b")
                    gff_b = bc2_pool.tile([P, E], F32, tag="gffb")
                    bff_b = bc2_pool.tile([P, E], F32, tag="bffb")
                    g2_b = bc2_pool.tile([P, E], F32, tag="g2b")
                    be2_b = bc2_pool.tile([P, E], F32, tag="be2b")
                    nc.gpsimd.dma_start(b2_b[:], _bcast_ap(b2, E))
                    nc.gpsimd.dma_start(gff_b[:], _bcast_ap(gff, E))
                    nc.gpsimd.dma_start(bff_b[:], _bcast_ap(bff, E))
                    nc.gpsimd.dma_start(g2_b[:], _bcast_ap(g2, E))
                    nc.gpsimd.dma_start(be2_b[:], _bcast_ap(be2, E))

                    def ln_tile(src_ap, dst_ap, g_b, b_b, nm):
                        stats = st2_pool.tile([P, 2, 6], F32, tag="st2",
                                              name=f"r{rep}_st2_{nm}")
                        mv = st2_pool.tile([P, 2], F32, tag="mv2",
                                           name=f"r{rep}_mv2_{nm}")
                        nc.vector.bn_stats(stats[:, 0, :], src_ap[:, 0:512])
                        nc.vector.bn_stats(stats[:, 1, :], src_ap[:, 512:1024])
                        nc.vector.bn_aggr(mv[:], stats[:])
                        rstd = st2_pool.tile([P, 1], F32, tag="rstd2",
                                             name=f"r{rep}_rstd2_{nm}")
                        nc.scalar.activation(rstd[:], mv[:, 1:2], AF.Sqrt,
                                             bias=eps_t[:])
                        nc.vector.reciprocal(rstd[:], rstd[:])
                        nc.vector.tensor_scalar(dst_ap, src_ap, mv[:, 0:1],
                                                rstd[:], ALU.subtract,
                                                ALU.mult)
                        nc.vector.tensor_tensor(dst_ap, dst_ap, g_b[:],
                                                ALU.mult)
                        nc.vector.tensor_tensor(dst_ap, dst_ap, b_b[:],
                                                ALU.add)

                    for m in range(NS):
                        ps = ps_2.tile([P, 1024], F32, tag="o",
                                       name=f"r{rep}_ps2_{m}")
                        half, mm = (0, m) if m < 4 else (1, m - 4)
                        for c in range(NF):
                            for n in range(2):
                                nc.tensor.matmul(
                                    ps[:, n * 512:(n + 1) * 512],
                                    hT[c][half][:, mm * P:(mm + 1) * P],
                                    w2_sb[c][:, n * 512:(n + 1) * 512],
                                    start=(c == 0), stop=(c == NF - 1))
                        ffb = ff_pool.tile([P, E], F32, tag="ffb",
                                           name=f"r{rep}_ffb_{m}")
                        nc.vector.tensor_tensor(ffb[:], ps[:], b2_b[:],
                                                ALU.add)
                        ln_tile(ffb[:], ffb[:], gff_b, bff_b, f"ff_{m}")
                        u2 = u2_pool.tile([P, E], F32, tag="u2",
                                          name=f"r{rep}_u2_{m}")
                        ln_tile(ffb[:], u2[:], g2_b, be2_b, f"l2_{m}")
                        ot = out_pool.tile([P, E], F32, tag="ot",
                                           name=f"r{rep}_ot_{m}")
                        nc.vector.tensor_tensor(ot[:], u2[:], a1bf[m][:],
                                                ALU.add)
                        nc.sync.dma_start(out_c[m], ot[:])

    _split_sync_waits(nc)
    nc.finalize()
    return nc


_NC = {}


def _get_nc(reps=1, mode="full"):
    key = (reps, mode)
    if key not in _NC:
        _NC[key] = build(reps, mode)
    return _NC[key]


def make_in_maps(inputs):
    bf = ml_dtypes.bfloat16
    x = np.ascontiguousarray(np.asarray(inputs["x"], dtype=np.float32))
    shared = {
        "wq": np.ascontiguousarray(np.asarray(inputs["Wq"], np.float32).astype(bf)),
        "wk": np.ascontiguousarray(np.asarray(inputs["Wk"], np.float32).astype(bf)),
        "wv": np.ascontiguousarray(np.asarray(inputs["Wv"], np.float32).astype(bf)),
        "w1": np.ascontiguousarray(np.asarray(inputs["W1"], np.float32).astype(bf)),
        "w2": np.ascontiguousarray(np.asarray(inputs["W2"], np.float32).astype(bf)),
        "bqt": np.ascontiguousarray(
            np.asarray(inputs["bq"], np.float32).reshape(NE, P).T),
        "bkt": np.ascontiguousarray(
            np.asarray(inputs["bk"], np.float32).reshape(NE, P).T),
        "bv": np.asarray(inputs["bv"], np.float32),
        "b1t": np.ascontiguousarray(
            np.asarray(inputs["b1"], np.float32).reshape(NF, P).T),
        "b2": np.asarray(inputs["b2"], np.float32),
        "g1": np.asarray(inputs["ln1_g"], np.float32),
        "be1": np.asarray(inputs["ln1_b"], np.float32),
        "gff": np.asarray(inputs["ln_ff_g"], np.float32),
        "bff": np.asarray(inputs["ln_ff_b"], np.float32),
        "g2": np.asarray(inputs["ln2_g"], np.float32),
        "be2": np.asarray(inputs["ln2_b"], np.float32),
    }
    in_maps = []
    for i in range(B):
        m = dict(shared)
        m["x"] = np.ascontiguousarray(x[i])
        m["xT"] = np.ascontiguousarray(x[i].T.astype(bf))
        in_maps.append(m)
    return in_maps


def kernel(**inputs):
    nc = _get_nc()
    in_maps = make_in_maps(inputs)
    res = run_bass_kernel_spmd(nc, in_maps, list(range(B)))
    return np.stack([res.results[i]["out"] for i in range(B)], axis=0)


# revision 35
# speedup vs baseline: 1.9168x; 1.1745x over previous
"""Trainium2 Bass kernel for nn_Block_52527450030210 (dense transformer block).

B=8, S=1024, E=1024, H=16 heads (D=64), F=4096. Data-parallel: batch element i
runs on core i (no collectives). Matmuls in bf16 with fp32 PSUM accumulation;
LayerNorms/softmax in fp32. Softmax is over the QUERY axis (dim=-2), so scores
are computed transposed ([k, q] layout) making the softmax reduction a
free-axis reduction, and 1/Z folds into v (Z is per contraction-index k).

v2 schedule: q/k projections are software-pipelined into the attention
k-tile loop to keep the PE array dense (HAM stays warm), softmax exps are
[128,1024]-wide (half the ACT instruction overhead), all transposes go
through the DMA xbar instead of the PE+ACT path, LN1 statistics are
accumulated incrementally during attention, FFN weights prefetch on the
SWDGE queue in halves sized to fit SBUF, and FFN1/FFN2 form one continuous
PE stream. adn1 stays in SBUF (bf16) instead of round-tripping through DRAM.

Self-contained: hardcodes shapes, includes the walrus single-sync-wait
workaround (this container's walrus accepts only one sync-wait per
instruction; Tile emits several, so extra waits are hoisted onto same-engine
NoOps).
"""

import numpy as np
import ml_dtypes

import concourse.bass as bass
import concourse.mybir as mybir
import concourse.tile as tile
from concourse.bass_utils import run_bass_kernel_spmd
from concourse.vector_clock import ScopedClock
from contextlib import ExitStack

F32 = mybir.dt.float32
BF16 = mybir.dt.bfloat16
AF = mybir.ActivationFunctionType
ALU = mybir.AluOpType

B, S, E, H, D, F = 8, 1024, 1024, 16, 64, 4096
P = 128
NE = E // P   # 8 e-chunks
NS = S // P   # 8 s-tiles
NF = F // P   # 32 f-tiles
EPS = 1e-5

# ---------------------------------------------------------------- waitfix ---

_wf_counter = [0]


def _wait_nop(nc, engine, wait, debug):
    _wf_counter[0] += 1
    nop = mybir.InstNoOp(
        name=f"I-wsplit-{_wf_counter[0]}", ins=[], outs=[], debug=debug,
        bass_nofuse=True,
    )
    nop.engine = engine
    nop.sync_info = mybir.SyncInfo(on_wait=[wait], on_update=[])
    nc.register_instruction(nop, overwrite=True)
    return nop


def _split_sync_waits(nc):
    for _name, bb in nc.bb_map.items():
        if not hasattr(bb, "instructions"):
            bb = bb.bb
        il = bb.instructions
        changed = False
        new = []
        for inst in il:
            si = inst.sync_info
            if si is not None and si.on_wait and len(si.on_wait) > 1:
                waits = list(si.on_wait)
                for w in waits[:-1]:
                    new.append(_wait_nop(nc, inst.engine, w, inst.debug))
                si.on_wait = waits[-1:]
                changed = True
            new.append(inst)
        if changed:
            bb.instructions = new


def _patched_drain_and_barrier(self, tick_clock, wait_clock):
    nop0 = self.nc.sync.nop(nofuse=True, hint="tail_wait")
    wait_clock.add_sem_waits(nop0.ins, ScopedClock({None: tick_clock.global_clock}))
    si = nop0.ins.sync_info
    waits = list(si.on_wait) if si and si.on_wait else []
    if len(waits) > 1:
        si.on_wait = waits[:1]
        rest = waits[1:]
        while rest:
            nop = self.nc.sync.nop(nofuse=True, hint="tail_wait")
            nop.ins.sync_info = mybir.SyncInfo(on_wait=rest[:1], on_update=[])
            rest = rest[1:]
    self.nc.sync.drain()
    self.nc.all_engine_barrier()
    assert self.sems is not None
    popped = self.nc._tile_sem_poison_stack.pop()
    assert popped is self._sem_poison
    self.nc.clear_and_free_semaphores(list(self.sems.allocated().values()))
    self.nc.all_engine_barrier()


tile.TileContext._drain_and_barrier = _patched_drain_and_barrier

# ----------------------------------------------------------------- build -----


def _bcast_ap(dram_t, n):
    """AP that DMA-broadcasts a [n] DRAM vector to [128, n] (partition step 0)."""
    return bass.AP(tensor=dram_t, offset=0, ap=[[0, P], [1, n]])


def build(reps=1, mode="full"):
    nc = bass.Bass()

    xT = nc.dram_tensor("xT", [E, S], BF16, kind="ExternalInput")
    x_f = nc.dram_tensor("x", [S, E], F32, kind="ExternalInput")
    wq = nc.dram_tensor("wq", [E, E], BF16, kind="ExternalInput")
    wk = nc.dram_tensor("wk", [E, E], BF16, kind="ExternalInput")
    wv = nc.dram_tensor("wv", [E, E], BF16, kind="ExternalInput")
    w1 = nc.dram_tensor("w1", [E, F], BF16, kind="ExternalInput")
    w2 = nc.dram_tensor("w2", [F, E], BF16, kind="ExternalInput")
    bqt = nc.dram_tensor("bqt", [P, NE], F32, kind="ExternalInput")
    bkt = nc.dram_tensor("bkt", [P, NE], F32, kind="ExternalInput")
    bv = nc.dram_tensor("bv", [E], F32, kind="ExternalInput")
    b1t = nc.dram_tensor("b1t", [P, NF], F32, kind="ExternalInput")
    b2 = nc.dram_tensor("b2", [E], F32, kind="ExternalInput")
    g1 = nc.dram_tensor("g1", [E], F32, kind="ExternalInput")
    be1 = nc.dram_tensor("be1", [E], F32, kind="ExternalInput")
    gff = nc.dram_tensor("gff", [E], F32, kind="ExternalInput")
    bff = nc.dram_tensor("bff", [E], F32, kind="ExternalInput")
    g2 = nc.dram_tensor("g2", [E], F32, kind="ExternalInput")
    be2 = nc.dram_tensor("be2", [E], F32, kind="ExternalInput")

    xT_c = xT.rearrange("(c p) s -> c p s", p=P)
    x_c = x_f.rearrange("(m p) e -> m p e", p=P)
    wq_c = wq.rearrange("(c p) e -> c p e", p=P)
    wk_c = wk.rearrange("(c p) e -> c p e", p=P)
    wv_c = wv.rearrange("(c p) e -> c p e", p=P)
    w1_c = w1.rearrange("(c p) f -> c p f", p=P)
    w2_c = w2.rearrange("(c p) e -> c p e", p=P)

    with tile.TileContext(nc) as tc:
      for rep in range(reps):
        out_d = nc.dram_tensor("out" if rep == 0 else f"out_r{rep}",
                               [S, E], F32, kind="ExternalOutput")
        out_c = out_d.rearrange("(m p) e -> m p e", p=P)
        with ExitStack() as top:
            const = top.enter_context(tc.tile_pool(name="const", bufs=1))
            eps_t = const.tile([P, 1], F32)
            nc.vector.memset(eps_t[:], EPS)
            bq_sb = const.tile([P, NE], F32)
            bk_sb = const.tile([P, NE], F32)
            b1_sb = const.tile([P, NF], F32)
            nc.sync.dma_start(bq_sb[:], bqt[:, :])
            nc.sync.dma_start(bk_sb[:], bkt[:, :])
            nc.sync.dma_start(b1_sb[:], b1t[:, :])

            # residual (bf16) and its transpose live across the whole rep
            a1_pool = top.enter_context(tc.tile_pool(name="a1p", bufs=NS))
            a1T_pool = top.enter_context(tc.tile_pool(name="a1Tp", bufs=2))
            a1bf = [a1_pool.tile([P, E], BF16, tag="a1", name=f"r{rep}_a1_{m}")
                    for m in range(NS)]
            # adn1T split by s-half so FFN1 n=0 only depends on LN1 of m=0..3;
            # [P, c, s] group-fold layout filled by one xbar transpose per m
            a1T = [a1T_pool.tile([P, NE, S // 2], BF16, tag="a1T",
                                 name=f"r{rep}_a1T_{n}") for n in range(2)]

            # ---------------------------------------------- attention ----
            with ExitStack() as attn:
                att_pool = attn.enter_context(tc.tile_pool(name="attp",
                                                           bufs=1))
                st_pool = attn.enter_context(tc.tile_pool(name="stp", bufs=NS))
                bc1_pool = attn.enter_context(tc.tile_pool(name="bc1p",
                                                           bufs=1))
                v_pool = attn.enter_context(tc.tile_pool(name="vp", bufs=NS))
                qk_pool = attn.enter_context(tc.tile_pool(name="qkp", bufs=6))
                exp_pool = attn.enter_context(tc.tile_pool(name="expp",
                                                           bufs=6))
                small_pool = attn.enter_context(tc.tile_pool(name="smallp",
                                                             bufs=8))
                wqkj_pool = attn.enter_context(tc.tile_pool(name="wqkjp",
                                                            bufs=6))
                attjT_pool = attn.enter_context(tc.tile_pool(name="attjTp",
                                                             bufs=1))
                u_pool = attn.enter_context(tc.tile_pool(name="up", bufs=1))

                # att in [s, e] layout: one tile, group-fold m = dim 1
                att_all = att_pool.tile([P, NS, E], BF16,
                                        name=f"r{rep}_att_all")
                st1 = [st_pool.tile([P, 4, 6], F32, tag="st1",
                                    name=f"r{rep}_st1_{m}")
                       for m in range(NS)]

                bv_b = bc1_pool.tile([P, E], BF16, tag="bvb")
                g1_b = bc1_pool.tile([P, E], BF16, tag="g1b")
                be1_b = bc1_pool.tile([P, E], BF16, tag="be1b")
                nc.gpsimd.dma_start(bv_b[:], _bcast_ap(bv, E))
                nc.gpsimd.dma_start(g1_b[:], _bcast_ap(g1, E))
                nc.gpsimd.dma_start(be1_b[:], _bcast_ap(be1, E))

                scopeA = ExitStack()      # xT (freed after last qk)
                attn.enter_context(scopeA)
                xT_pool = scopeA.enter_context(tc.tile_pool(name="xTp",
                                                            bufs=1))

                # per-j [P, c, 128] slices of wq/wk; full tensors stay in DRAM
                wq_j = wq.rearrange("(c p) e -> p c e", p=P)
                wk_j = wk.rearrange("(c p) e -> p c e", p=P)

                # wj(0) first so qk(0) isn't queued behind the 6MB below
                wj0 = []
                for di, w_dram in enumerate((wq_j, wk_j)):
                    wj = wqkj_pool.tile([P, NE, P], BF16, tag="wqkj",
                                        name=f"r{rep}_wj_0_{di}")
                    nc.sync.dma_start(wj[:], w_dram[:, :, 0:P])
                    wj0.append(wj)

                # xT/wv streamed per chunk so dependent matmuls overlap DMA
                xT_all = xT_pool.tile([P, NE, S], BF16, name=f"r{rep}_xT")
                wv_scope = ExitStack()
                attn.enter_context(wv_scope)
                wv_pool = wv_scope.enter_context(tc.tile_pool(name="wvp",
                                                              bufs=1))
                wv_all = wv_pool.tile([P, NE, E], BF16, name=f"r{rep}_wv")
                for c in range(NE):
                    nc.sync.dma_start(xT_all[:, c, :], xT_c[c])
                    nc.sync.dma_start(wv_all[:, c, :], wv_c[c])
                for m in range(NS):
                    nc.gpsimd.dma_start(a1bf[m][:], x_c[m])

                ps_big = attn.enter_context(
                    tc.tile_pool(name="ps_big", bufs=3, space="PSUM"))
                ps_av = attn.enter_context(
                    tc.tile_pool(name="ps_av", bufs=1, space="PSUM"))

                def emit_qk(j, wj_pre=None):
                    """qT[j], kT[j] (bf16 [P,S]) via shared ps_big pool."""
                    qt = qk_pool.tile([P, S], BF16, tag="qkT",
                                      name=f"r{rep}_qT_{j}")
                    kt = qk_pool.tile([P, S], BF16, tag="qkT",
                                      name=f"r{rep}_kT_{j}")
                    for di, (dst, w_dram, bias) in enumerate(
                            ((qt, wq_j, bq_sb), (kt, wk_j, bk_sb))):
                        if wj_pre is not None:
                            wj = wj_pre[di]
                        else:
                            wj = wqkj_pool.tile([P, NE, P], BF16, tag="wqkj",
                                                name=f"r{rep}_wj_{j}_{di}")
                            nc.sync.dma_start(wj[:],
                                              w_dram[:, :, j * P:(j + 1) * P])
                        ps = ps_big.tile([P, 1024], F32, tag="big",
                                         name=f"r{rep}_psqk_{j}_{di}")
                        for n in range(2):
                            for c in range(NE):
                                nc.tensor.matmul(
                                    ps[:, n * 512:(n + 1) * 512],
                                    wj[:, c, :],
                                    xT_all[:, c, n * 512:(n + 1) * 512],
                                    start=(c == 0), stop=(c == NE - 1))
                        nc.vector.tensor_scalar_add(dst[:], ps[:],
                                                    bias[:, j:j + 1])
                    return qt, kt

                def emit_v(m):
                    vt = v_pool.tile([P, E], BF16, tag="v", name=f"r{rep}_v_{m}")
                    ps = ps_big.tile([P, 1024], F32, tag="big",
                                     name=f"r{rep}_psv_{m}")
                    for n in range(2):
                        for c in range(NE):
                            nc.tensor.matmul(
                                ps[:, n * 512:(n + 1) * 512],
                                xT_all[:, c, m * P:(m + 1) * P],
                                wv_all[:, c, n * 512:(n + 1) * 512],
                                start=(c == 0), stop=(c == NE - 1))
                    nc.vector.tensor_tensor(vt[:], ps[:], bv_b[:], ALU.add)
                    return vt

                qk_sb = {0: emit_qk(0, wj_pre=wj0)}
                v_sb = [emit_v(m) for m in range(3)]

                # qk for later j: spread as 4-MM quarter-groups, one per kc
                # step, so the PE never crowds out the ACT-paced softmax.
                qk_pending = []   # FIFO of (jt, di, qd)
                qk_state = {}

                def start_qk(jt):
                    st = {
                        "qt": qk_pool.tile([P, S], BF16, tag="qkT",
                                           name=f"r{rep}_qT_{jt}"),
                        "kt": qk_pool.tile([P, S], BF16, tag="qkT",
                                           name=f"r{rep}_kT_{jt}"),
                    }
                    qk_state[jt] = st
                    qk_pending.extend((jt, di, qd)
                                      for di in range(2) for qd in range(4))

                def emit_qk_quarter(jt, di, qd):
                    st = qk_state[jt]
                    w_dram, bias = ((wq_j, bq_sb), (wk_j, bk_sb))[di]
                    if qd == 0:
                        wj = wqkj_pool.tile([P, NE, P], BF16, tag="wqkj",
                                            name=f"r{rep}_wj_{jt}_{di}")
                        nc.sync.dma_start(wj[:],
                                          w_dram[:, :, jt * P:(jt + 1) * P])
                        st["wj"] = wj
                        st["ps"] = ps_big.tile([P, 1024], F32, tag="big",
                                               name=f"r{rep}_psqk_{jt}_{di}")
                    n, ch = qd // 2, qd % 2
                    ps = st["ps"]
                    for c in range(4 * ch, 4 * ch + 4):
                        nc.tensor.matmul(
                            ps[:, n * 512:(n + 1) * 512],
                            st["wj"][:, c, :],
                            xT_all[:, c, n * 512:(n + 1) * 512],
                            start=(c == 0), stop=(c == NE - 1))
                    if qd == 3:
                        dst = st["qt"] if di == 0 else st["kt"]
                        nc.vector.tensor_scalar_add(dst[:], ps[:],
                                                    bias[:, jt:jt + 1])
                        del st["ps"], st["wj"]

                start_qk(1)

                def emit_sc_exp(j, kc, h, qt, kt):
                    o = 64 * h
                    scps = ps_big.tile([P, 1024], F32, tag="big",
                                       name=f"r{rep}_sc_{j}_{kc}_{h}")
                    for n in range(2):
                        nc.tensor.matmul(
                            scps[:, n * 512:(n + 1) * 512],
                            kt[o:o + 64, kc * P:(kc + 1) * P],
                            qt[o:o + 64, n * 512:(n + 1) * 512],
                            start=True, stop=True, tile_position=(o, 0))
                    expt = exp_pool.tile([P, 1024], BF16, tag="expt",
                                         name=f"r{rep}_ex_{j}_{kc}_{h}")
                    zt = small_pool.tile([P, 1], F32, tag="z",
                                         name=f"r{rep}_z_{j}_{kc}_{h}")
                    nc.scalar.activation(expt[:], scps[:], AF.Exp,
                                         scale=0.1, accum_out=zt[:])
                    nc.vector.reciprocal(zt[:], zt[:])
                    vsc = small_pool.tile([P, 64], BF16, tag="vsc",
                                          name=f"r{rep}_vs_{j}_{kc}_{h}")
                    head = 2 * j + h
                    nc.vector.tensor_scalar_mul(
                        vsc[:], v_sb[kc][:, head * 64:(head + 1) * 64], zt[:])
                    return expt, vsc

                def emit_av(avp, kc, h, expt, vsc):
                    o = 64 * h
                    for n in range(2):
                        nc.tensor.matmul(
                            avp[o:o + 64, n * 512:(n + 1) * 512],
                            vsc[:], expt[:, n * 512:(n + 1) * 512],
                            start=(kc == 0), stop=(kc == NS - 1),
                            tile_position=(0, o))

                pending_stats = []   # (m, jpair) bn_stats to spread over kc

                def epilogue(j, avp):
                    """Drain avp(j) to att_all cols j (runs inside j+1 kc0)."""
                    attjT = attjT_pool.tile([P, S], BF16, tag="attjT",
                                            name=f"r{rep}_attjT_{j}")
                    nc.vector.tensor_copy(attjT[:], avp[:])
                    nc.sync.dma_start_transpose(
                        att_all[:, :, j * P:(j + 1) * P], attjT[:])
                    if j % 2 == 1:
                        if j < NE - 1:
                            pending_stats.extend(
                                (m, j // 2) for m in range(NS))
                        else:
                            for m in range(NS):
                                nc.vector.bn_stats(
                                    st1[m][:, j // 2, :],
                                    att_all[:, m, (j - 1) * P:(j + 1) * P])

                pend = None   # (avp, kc, [(h, expt, vsc)...], j)
                for j in range(NE):
                    qt, kt = (qk_sb.pop(0) if j == 0
                              else (qk_state[j]["qt"], qk_state[j]["kt"]))
                    if j + 2 < NE:
                        start_qk(j + 2)
                    avp = ps_av.tile([P, 1024], F32, tag="av",
                                     name=f"r{rep}_av_{j}")
                    for kc in range(NS):
                        if j == 0:
                            while len(v_sb) < min(kc + 4, NS):
                                v_sb.append(emit_v(len(v_sb)))
                            if kc == 5:
                                wv_scope.close()
                        cur = []
                        for h in range(2):
                            cur.append((h,) + emit_sc_exp(j, kc, h, qt, kt))
                        if pend is not None:
                            pavp, pkc, items, pj = pend
                            for h, expt, vsc in items:
                                emit_av(pavp, pkc, h, expt, vsc)
                            if pkc == NS - 1:
                                epilogue(pj, pavp)
                        pend = (avp, kc, cur, j)
                        if qk_pending:
                            emit_qk_quarter(*qk_pending.pop(0))
                        if pending_stats:
                            m, jp = pending_stats.pop()
                            nc.vector.bn_stats(
                                st1[m][:, jp, :],
                                att_all[:, m, (2 * jp) * P:(2 * jp + 2) * P])
                    # residual prep (bf16): a1bf[j] = x[j] + be1
                    nc.vector.tensor_tensor(a1bf[j][:], a1bf[j][:], be1_b[:],
                                            ALU.add)
                    if j == NE - 2:
                        scopeA.close()   # free xT for w1 prefetch
                pavp, pkc, items, pj = pend
                for h, expt, vsc in items:
                    emit_av(pavp, pkc, h, expt, vsc)
                epilogue(pj, pavp)

                # -------------------------------------- LN1 + adn1T ----
                mv1, rstd1 = [], []
                for m in range(NS):
                    mv = st_pool.tile([P, 2], F32, tag="mv1",
                                      name=f"r{rep}_mv1_{m}")
                    nc.vector.bn_aggr(mv[:], st1[m][:])
                    rstd = st_pool.tile([P, 1], F32, tag="rstd1",
                                        name=f"r{rep}_rstd1_{m}")
                    nc.scalar.activation(rstd[:], mv[:, 1:2], AF.Sqrt,
                                         bias=eps_t[:])
                    nc.vector.reciprocal(rstd[:], rstd[:])
                    mv1.append(mv)
                    rstd1.append(rstd)
                for m in range(NS):
                    u = u_pool.tile([P, E], BF16, tag="u", name=f"r{rep}_u_{m}")
                    nc.vector.tensor_scalar(u[:], att_all[:, m, :],
                                            mv1[m][:, 0:1], rstd1[m][:],
                                            ALU.subtract, ALU.mult)
                    nc.vector.tensor_tensor(u[:], u[:], g1_b[:], ALU.mult)
                    nc.vector.tensor_tensor(a1bf[m][:], a1bf[m][:], u[:],
                                            ALU.add)
                    nc.sync.dma_start_transpose(
                        a1T[m // 4][:, :, (m % 4) * P:(m % 4 + 1) * P],
                        a1bf[m][:])

            # ------------------------------------------------ FFN ----
            with ExitStack() as ffn:
                hT_pool = ffn.enter_context(tc.tile_pool(name="hTp",
                                                         bufs=2 * NF))
                ps_f = ffn.enter_context(
                    tc.tile_pool(name="ps_f", bufs=4, space="PSUM"))
                w2a_pool = ffn.enter_context(tc.tile_pool(name="w2ap",
                                                          bufs=NF // 2))
                scopeW1 = ExitStack()     # w1 halves; closed after FFN1
                ffn.enter_context(scopeW1)
                w1a_pool = scopeW1.enter_context(
                    tc.tile_pool(name="w1ap", bufs=NE))
                w1b_pool = scopeW1.enter_context(
                    tc.tile_pool(name="w1bp", bufs=NE))
                w1a_sb, w1b_sb, w2_sb = [], [], []
                for c in range(NE):
                    t = w1a_pool.tile([P, F // 2], BF16, tag="w1a",
                                      name=f"r{rep}_w1a_{c}")
                    nc.gpsimd.dma_start(t[:], w1_c[c][:, 0:F // 2])
                    w1a_sb.append(t)
                for c in range(NE):
                    t = w1b_pool.tile([P, F // 2], BF16, tag="w1b",
                                      name=f"r{rep}_w1b_{c}")
                    nc.gpsimd.dma_start(t[:], w1_c[c][:, F // 2:F])
                    w1b_sb.append(t)
                for c in range(NF // 2):
                    t = w2a_pool.tile([P, E], BF16, tag="w2a",
                                      name=f"r{rep}_w2_{c}")
                    nc.gpsimd.dma_start(t[:], w2_c[c])
                    w2_sb.append(t)

                # hT split by s-half (FFN2 m<4 only needs the n=0 half)
                hT = [[hT_pool.tile([P, S // 2], BF16, tag="hT",
                                    name=f"r{rep}_hT_{f}_{n}") for n in range(2)]
                      for f in range(NF)]

                for n in range(2):
                    for f in range(NF):
                        ps = ps_f.tile([P, 512], F32, tag="f",
                                       name=f"r{rep}_psf_{f}_{n}")
                        w_half = w1a_sb if f < NF // 2 else w1b_sb
                        fo = f if f < NF // 2 else f - NF // 2
                        for c in range(NE):
                            nc.tensor.matmul(
                                ps[:],
                                w_half[c][:, fo * P:(fo + 1) * P],
                                a1T[n][:, c, :],
                                start=(c == 0), stop=(c == NE - 1))
                        nc.scalar.activation(hT[f][n][:], ps[:], AF.Gelu,
                                             bias=b1_sb[:, f:f + 1])

                scopeW1.close()   # free w1 halves; a1T no longer needed
                with ExitStack() as ffn2:
                    ps_2 = ffn2.enter_context(
                        tc.tile_pool(name="ps_2", bufs=2, space="PSUM"))
                    w2b_pool = ffn2.enter_context(tc.tile_pool(name="w2bp",
                                                               bufs=NF // 2))
                    bc2_pool = ffn2.enter_context(tc.tile_pool(name="bc2p",
                                                               bufs=1))
                    ff_pool = ffn2.enter_context(tc.tile_pool(name="ffp",
                                                              bufs=2))
                    u2_pool = ffn2.enter_context(tc.tile_pool(name="u2p",
                                                              bufs=2))
                    st2_pool = ffn2.enter_context(tc.tile_pool(name="st2p",
                                                               bufs=6))
                    out_pool = ffn2.enter_context(tc.tile_pool(name="outp",
                                                               bufs=2))
                    for c in range(NF // 2, NF):
                        t = w2b_pool.tile([P, E], BF16, tag="w2b",
                                          name=f"r{rep}_w2_{c}")
                        nc.gpsimd.dma_start(t[:], w2_c[c])
                        w2_sb.append(t)

                    b2_b = bc2_pool.tile([P, E], F32, tag="b2b")
                    gff_b = bc2_pool.tile([P, E], F32, tag="gffb")
                    bff_b = bc2_pool.tile([P, E], F32, tag="bffb")
                    g2_b = bc2_pool.tile([P, E], F32, tag="g2b")
                    be2_b = bc2_pool.tile([P, E], F32, tag="be2b")
                    nc.gpsimd.dma_start(b2_b[:], _bcast_ap(b2, E))
                    nc.gpsimd.dma_start(gff_b[:], _bcast_ap(gff, E))
                    nc.gpsimd.dma_start(bff_b[:], _bcast_ap(bff, E))
                    nc.gpsimd.dma_start(g2_b[:], _bcast_ap(g2, E))
                    nc.gpsimd.dma_start(be2_b[:], _bcast_ap(be2, E))

                    def ln_tile(src_ap, dst_ap, g_b, b_b, nm):
                        stats = st2_pool.tile([P, 2, 6], F32, tag="st2",
                                              name=f"r{rep}_st2_{nm}")
                        mv = st2_pool.tile([P, 2], F32, tag="mv2",
                                           name=f"r{rep}_mv2_{nm}")
                        nc.vector.bn_stats(stats[:, 0, :], src_ap[:, 0:512])
                        nc.vector.bn_stats(stats[:, 1, :], src_ap[:, 512:1024])
                        nc.vector.bn_aggr(mv[:], stats[:])
                        rstd = st2_pool.tile([P, 1], F32, tag="rstd2",
                                             name=f"r{rep}_rstd2_{nm}")
                        nc.scalar.activation(rstd[:], mv[:, 1:2], AF.Sqrt,
                                             bias=eps_t[:])
                        nc.vector.reciprocal(rstd[:], rstd[:])
                        nc.vector.tensor_scalar(dst_ap, src_ap, mv[:, 0:1],
                                                rstd[:], ALU.subtract,
                                                ALU.mult)
                        nc.vector.tensor_tensor(dst_ap, dst_ap, g_b[:],
                                                ALU.mult)
                        nc.vector.tensor_tensor(dst_ap, dst_ap, b_b[:],
                                                ALU.add)

                    for m in range(NS):
                        ps = ps_2.tile([P, 1024], F32, tag="o",
                                       name=f"r{rep}_ps2_{m}")
                        half, mm = (0, m) if m < 4 else (1, m - 4)
                        for c in range(NF):
                            for n in range(2):
                                nc.tensor.matmul(
                                    ps[:, n * 512:(n + 1) * 512],
                                    hT[c][half][:, mm * P:(mm + 1) * P],
                                    w2_sb[c][:, n * 512:(n + 1) * 512],
                                    start=(c == 0), stop=(c == NF - 1))
                        ffb = ff_pool.tile([P, E], F32, tag="ffb",
                                           name=f"r{rep}_ffb_{m}")
                        nc.vector.tensor_tensor(ffb[:], ps[:], b2_b[:],
                                                ALU.add)
                        ln_tile(ffb[:], ffb[:], gff_b, bff_b, f"ff_{m}")
                        u2 = u2_pool.tile([P, E], F32, tag="u2",
                                          name=f"r{rep}_u2_{m}")
                        ln_tile(ffb[:], u2[:], g2_b, be2_b, f"l2_{m}")
                        ot = out_pool.tile([P, E], F32, tag="ot",
                                           name=f"r{rep}_ot_{m}")
                        nc.vector.tensor_tensor(ot[:], u2[:], a1bf[m][:],
                                                ALU.add)
                        nc.sync.dma_start(out_c[m], ot[:])

    _split_sync_waits(nc)
    nc.finalize()
    return nc


_NC = {}


def _get_nc(reps=1, mode="full"):
    key = (reps, mode)
    if key not in _NC:
        _NC[key] = build(reps, mode)
    return _NC[key]


def make_in_maps(inputs):
    bf = ml_dtypes.bfloat16
    x = np.ascontiguousarray(np.asarray(inputs["x"], dtype=np.float32))
    shared = {
        "wq": np.ascontiguousarray(np.asarray(inputs["Wq"], np.float32).astype(bf)),
        "wk": np.ascontiguousarray(np.asarray(inputs["Wk"], np.float32).astype(bf)),
        "wv": np.ascontiguousarray(np.asarray(inputs["Wv"], np.float32).astype(bf)),
        "w1": np.ascontiguousarray(np.asarray(inputs["W1"], np.float32).astype(bf)),
        "w2": np.ascontiguousarray(np.asarray(inputs["W2"], np.float32).astype(bf)),
        "bqt": np.ascontiguousarray(
            np.asarray(inputs["bq"], np.float32).reshape(NE, P).T),
        "bkt": np.ascontiguousarray(
            np.asarray(inputs["bk"], np.float32).reshape(NE, P).T),
        "bv": np.asarray(inputs["bv"], np.float32),
        "b1t": np.ascontiguousarray(
            np.asarray(inputs["b1"], np.float32).reshape(NF, P).T),
        "b2": np.asarray(inputs["b2"], np.float32),
        "g1": np.asarray(inputs["ln1_g"], np.float32),
        "be1": np.asarray(inputs["ln1_b"], np.float32),
        "gff": np.asarray(inputs["ln_ff_g"], np.float32),
        "bff": np.asarray(inputs["ln_ff_b"], np.float32),
        "g2": np.asarray(inputs["ln2_g"], np.float32),
        "be2": np.asarray(inputs["ln2_b"], np.float32),
    }
    in_maps = []
    for i in range(B):
        m = dict(shared)
        m["x"] = np.ascontiguousarray(x[i])
        m["xT"] = np.ascontiguousarray(x[i].T.astype(bf))
        in_maps.append(m)
    return in_maps


def kernel(**inputs):
    nc = _get_nc()
    in_maps = make_in_maps(inputs)
    res = run_bass_kernel_spmd(nc, in_maps, list(range(B)))
    return np.stack([res.results[i]["out"] for i in range(B)], axis=0)
